# revision 30
# baseline (speedup 1.0000x reference)
"""Causal self-attention (B=4, T=2048, C=768, H=12) on 8 trn2 NeuronCores.

Sharding: 8 cores = 4 batches x 2 head-groups (6 heads each).
Each core: QKV projection for its 6 heads, causal attention, partial output
projection (row-parallel). Host sums the two partials per batch + b_proj.

Device-side dataflow (v2, fp8 DoubleRow):
  - x shipped as x^T fp8e4 (values ~N(0,1), well inside e4m3 range).
  - QKV weights host-scaled by 32 (avoids fp8 subnormals) and packed in
    contraction PAIRS for DoubleRow matmuls: lhsT/rhs APs [128, 2, M].
    Q,K come out x32; the 1/32^2 is folded into the exp scale (1/8192).
    V comes out x32 with a x32 ones column, so y = (32*sum z v)/(32*sum z)
    is exactly unscaled.
  - Causal mask: PE "preset" matmuls accumulate -245760 * [ones|L] into the
    masked PSUM region before the S matmul lands; exp(x/8192) then flushes
    those entries to exactly 0 in fp8. No vector-engine masking at all.
  - S^T computed per k-block into [128, 2, 512] PSUM pair tiles (2 banks);
    ONE exp activation per (k-pair, head) writes the fp8 z pair tile.
  - AV matmul in fp8 DoubleRow over k-block pairs; the V' ones column
    yields the softmax denominator for free. y normalized via
    reciprocal_approx_fast + gpsimd partition_broadcast + vector multiply.
  - Output projection in bf16, result DMA'd out as bf16 partials.
"""

import os
import sys
import types

sys.path.insert(0, "/opt/trn_rl_repo")

import ml_dtypes
import numpy as np

import concourse.bass as bass
import concourse.tile as tile
from concourse import bacc, mybir
from concourse.bass_utils import run_bass_kernel_spmd

B, T, C, H, D = 4, 2048, 768, 12, 64
N_CORES = 8
HPC = H // 2          # heads per core = 6
FQK = 2 * HPC * D     # 768 qk features per core
FV = HPC * D          # 384 v features per core
E = D + 1             # 65: head dim + ones column
EP = 68               # padded V' per-head width (dual-fp8 ldweights alignment)
CCH = C // 128        # 6 contraction chunks
CP = CCH // 2         # 3 contraction chunk pairs (DoubleRow)
QC = T // 512         # 4 query chunks of 512
F32 = mybir.dt.float32
BF16 = mybir.dt.bfloat16
FP8 = mybir.dt.float8e4
NPBF = ml_dtypes.bfloat16
NPF8 = ml_dtypes.float8_e4m3
DR = mybir.MatmulPerfMode.DoubleRow

WS = 32.0             # host weight scale (fp8 subnormal avoidance)
EXPS = 1.0 / (np.sqrt(D) * WS * WS)   # 1/(8*1024) = 1/8192
NEG = -30.0 * WS * WS * np.sqrt(D)    # -245760; exp(NEG*EXPS) == exp(-30)


def _install_ntff_hook():
    """The image's antenv lacks axon_hooks; inject it so trace=True works."""
    if "antenv.axon_hooks" in sys.modules:
        return
    try:
        import antenv
        mod = types.ModuleType("antenv.axon_hooks")
        _state = {"hook": None}
        mod.set_axon_ntff_profile_hook = lambda h: _state.__setitem__("hook", h)
        mod.get_axon_ntff_profile_hook = lambda: _state["hook"]
        sys.modules["antenv.axon_hooks"] = mod
        antenv.axon_hooks = mod
        from trn_agent_boot.trn_boot import _ntff_profile_via_ctypes
        mod.set_axon_ntff_profile_hook(
            _ntff_profile_via_ctypes("/opt/axon/libaxon_pjrt.so")
        )
    except Exception:
        pass


def _build_program():
    nc = bacc.Bacc(
        "TRN2",
        target_bir_lowering=False,
        debug=False,
        enable_asserts=False,
        num_devices=N_CORES,
    )
    xtd = nc.dram_tensor("xtd", [C, T], FP8, kind="ExternalInput").ap()
    xbd = nc.dram_tensor("xbd", [C, 512], BF16, kind="ExternalInput").ap()
    wqk = nc.dram_tensor("wqk", [CP * 128, 2 * FQK], FP8, kind="ExternalInput").ap()
    wv = nc.dram_tensor("wv", [CP * 128, 2 * HPC * EP], FP8, kind="ExternalInput").ap()
    wvb = nc.dram_tensor("wvb", [C, HPC * EP], BF16, kind="ExternalInput").ap()
    bqk = nc.dram_tensor("bqk", [FQK], F32, kind="ExternalInput").ap()
    bvb = nc.dram_tensor("bvb", [128, HPC * EP], F32, kind="ExternalInput").ap()
    wp = nc.dram_tensor("wp", [FV, C], BF16, kind="ExternalInput").ap()
    maskd = nc.dram_tensor("maskd", [128, 256], BF16, kind="ExternalInput").ap()
    idnd = nc.dram_tensor("idnd", [128, 128], BF16, kind="ExternalInput").ap()
    yp = nc.dram_tensor("yp", [T, C], BF16, kind="ExternalOutput").ap()
    dbg = None
    if os.environ.get("KDEBUG"):
        dbg = {
            "dqkt0": nc.dram_tensor("dqkt0", [128, T], BF16,
                                    kind="ExternalOutput").ap(),
            "dqkt3": nc.dram_tensor("dqkt3", [128, T], BF16,
                                    kind="ExternalOutput").ap(),
            "dvp0": nc.dram_tensor("dvp0", [128, 1024], FP8,
                                   kind="ExternalOutput").ap(),
            "dyts0": nc.dram_tensor("dyts0", [128, 512], BF16,
                                    kind="ExternalOutput").ap(),
            "dyts1": nc.dram_tensor("dyts1", [128, 512], BF16,
                                    kind="ExternalOutput").ap(),
            "dsp": nc.dram_tensor("dsp", [128, 1024], F32,
                                  kind="ExternalOutput").ap(),
            "dzt": nc.dram_tensor("dzt", [128, 1024], BF16,
                                  kind="ExternalOutput").ap(),
            "dyz": nc.dram_tensor("dyz", [EP, 512], F32,
                                  kind="ExternalOutput").ap(),
            "drc": nc.dram_tensor("drc", [1, 512], F32,
                                  kind="ExternalOutput").ap(),
        }

    with tile.TileContext(nc) as tc:
        _body(tc, nc, xtd, xbd, wqk, wv, wvb, bqk, bvb, wp, maskd, idnd, yp,
              dbg)

    nc.compile()
    return nc


def _body(tc, nc, xtd, xbd, wqk, wv, wvb, bqk, bvb, wp, maskd, idnd, yp,
          dbg=None):
    from contextlib import ExitStack

    with ExitStack() as es:
        persist = es.enter_context(tc.tile_pool(name="persist", bufs=1))
        # S pair tiles + QKV-proj pair tiles share one 2-bank-tile pool
        spp = es.enter_context(tc.tile_pool(name="spp", bufs=2, space="PSUM"))
        pvpp = es.enter_context(tc.tile_pool(name="pvpp", bufs=2, space="PSUM"))
        psyz = es.enter_context(tc.tile_pool(name="psyz", bufs=2, space="PSUM"))
        zpool = es.enter_context(tc.tile_pool(name="zpool", bufs=4))
        zpoolb = es.enter_context(tc.tile_pool(name="zpoolb", bufs=2))
        ypool = es.enter_context(tc.tile_pool(name="ypool", bufs=2))
        opool = es.enter_context(tc.tile_pool(name="opool", bufs=3))
        spool = es.enter_context(tc.tile_pool(name="spool", bufs=2))

        # ---- small constants first (scalar queue) so big loads start early
        maskc = persist.tile([128, 256], BF16, tag="maskc", name="maskc")
        nc.scalar.dma_start(maskc[:], maskd[:])
        idn = persist.tile([128, 128], BF16, tag="idn", name="idn")
        nc.scalar.dma_start(idn[:], idnd[:])
        bqk_sb = persist.tile([128, CCH], F32, tag="bqk", name="bqk_sb")
        nc.scalar.dma_start(bqk_sb[:], bqk.rearrange("(f p) -> p f", p=128))
        bvb_sb = persist.tile([128, HPC * EP], F32, tag="bvb", name="bvb_sb")
        nc.scalar.dma_start(bvb_sb[:], bvb[:])

        wqk_sb = [persist.tile([128, 2, FQK], FP8, tag=f"wqk{i}", name=f"wqk_sb{i}")
                  for i in range(CP)]
        # j-stride padded to 512 (dual-fp8 ISA: k-tile stride must be
        # 16-byte aligned)
        wv_sb = [persist.tile([128, 2, 512], FP8, tag=f"wv{i}", name=f"wv_sb{i}")
                 for i in range(CP)]
        wp_sb = [persist.tile([128, C], BF16, tag=f"wp{i}", name=f"wp_sb{i}")
                 for i in range(FV // 128)]
        for i in range(CP):
            for j in range(2):
                nc.gpsimd.dma_start(
                    wqk_sb[i][:, j, :],
                    wqk[i * 128:(i + 1) * 128, j * FQK:(j + 1) * FQK])
        for i in range(CP):
            for j in range(2):
                nc.gpsimd.dma_start(
                    wv_sb[i][:, j, 0:HPC * EP],
                    wv[i * 128:(i + 1) * 128, j * HPC * EP:(j + 1) * HPC * EP])
        for i in range(FV // 128):
            nc.gpsimd.dma_start(wp_sb[i][:], wp[i * 128:(i + 1) * 128, :])

        # x^T fp8, stored as chunk-pair tiles for DoubleRow
        xT = [persist.tile([128, 2, T], FP8, tag=f"xT{i}", name=f"xT{i}")
              for i in range(CP)]
        # QK^T bf16: tiles 0..2 hold Q^T (6 heads x 64), 3..5 hold K^T
        qkt = [persist.tile([128, T], BF16, tag=f"qkt{i}", name=f"qkt{i}")
               for i in range(CCH)]
        # V' fp8 pair tiles: [tok 128, kpair j, 6*65 feats] per k-block pair
        vp = [persist.tile([128, 2, 512], FP8, tag=f"vp{i}", name=f"vp{i}")
              for i in range(T // 256)]
        # bf16 V' for the first 4 token blocks (used by the q4=0 bf16 path)
        vpb = [persist.tile([128, HPC * EP], BF16, tag=f"vpb{i}", name=f"vpb{i}")
               for i in range(4)]
        # bf16 x^T (first 512 tokens) + bf16 wv for the q4=0 V projection
        xTb = [persist.tile([128, 512], BF16, tag=f"xTb{i}", name=f"xTb{i}")
               for i in range(CCH)]
        wvb_sb = [persist.tile([128, HPC * EP], BF16, tag=f"wvb{i}",
                               name=f"wvb_sb{i}") for i in range(CCH)]
        for i in range(CCH):
            nc.gpsimd.dma_start(wvb_sb[i][:], wvb[i * 128:(i + 1) * 128, :])
        for i in range(CCH):
            nc.gpsimd.dma_start(xTb[i][:], xbd[i * 128:(i + 1) * 128, :])

        def a_chunk(t4):
            # DMA this chunk's x^T columns (pre-transposed fp8 on host)
            for cp in range(CP):
                for j in range(2):
                    nc.sync.dma_start(
                        xT[cp][:, j, t4 * 512:(t4 + 1) * 512],
                        xtd[(2 * cp + j) * 128:(2 * cp + j + 1) * 128,
                            t4 * 512:(t4 + 1) * 512],
                    )

        def b_round(q4, r):
            # Q^T (ft=r) and K^T (ft=r+3) for head pair 2r, 2r+1
            for ft in (r, r + 3):
                ps = pvpp.tile([128, 512], F32, tag="pvpp", name="ps",
                               padded_shape=[128, 512])
                for cp in range(CP):
                    nc.tensor.matmul(
                        ps[:],
                        wqk_sb[cp][:, :, ft * 128:(ft + 1) * 128],
                        xT[cp][:, :, q4 * 512:(q4 + 1) * 512],
                        start=(cp == 0),
                        stop=(cp == CP - 1),
                        perf_mode=DR,
                    )
                nc.vector.tensor_scalar_add(
                    qkt[ft][:, q4 * 512:(q4 + 1) * 512],
                    ps[:],
                    bqk_sb[:, ft:ft + 1],
                )

        def c_chunk(t4):
            # V' fp8 pair tiles for 4 token blocks
            for j in range(4):
                tt = t4 * 4 + j
                pv = pvpp.tile([128, HPC * EP], F32, tag="pvpp", name="pv")
                for cp in range(CP):
                    nc.tensor.matmul(
                        pv[:],
                        xT[cp][:, :, tt * 128:(tt + 1) * 128],
                        wv_sb[cp][:, :, 0:HPC * EP],
                        start=(cp == 0),
                        stop=(cp == CP - 1),
                        perf_mode=DR,
                    )
                nc.vector.tensor_add(
                    vp[tt // 2][:, tt % 2, 0:HPC * EP], pv[:], bvb_sb[:])

        def ccb_unit(tt):
            # bf16 V' for one of token blocks 0..3 (q4=0 bf16 AV path)
            pv = pvpp.tile([128, HPC * EP], F32, tag="pvpp", name="pvb")
            for cc in range(CCH):
                nc.tensor.matmul(
                    pv[:],
                    xTb[cc][:, tt * 128:(tt + 1) * 128],
                    wvb_sb[cc][:],
                    start=(cc == 0),
                    stop=(cc == CCH - 1),
                )
            nc.vector.tensor_add(vpb[tt][:], pv[:], bvb_sb[:])

        def kt_slice(h, kb):
            return qkt[3 + h // 2][(h % 2) * 64:(h % 2) * 64 + 64,
                                   kb * 128:(kb + 1) * 128]

        def q_slice(h, q4, c0, c1):
            return qkt[h // 2][(h % 2) * 64:(h % 2) * 64 + 64,
                               q4 * 512 + c0:q4 * 512 + c1]

        def dummy(n=256):
            pv = pvpp.tile([128, 512], F32, tag="pvpp", name="dum")
            nc.tensor.matmul(pv[:, 0:n], idn[:], xTb[0][:, 0:n],
                             start=True, stop=True)

        def attn_heads(q4, yts, heads, fill=None):
            nkp = 2 * q4 + 2
            yzs = {h: psyz.tile([EP, 512], F32, tag="yz", name=f"yz{h}")
                   for h in heads}
            prev_av = None
            for kp in range(nkp):
                po = max(0, kp * 256 - q4 * 512)  # pair col offset
                for h in heads:
                    sp = spp.tile([128, 2, 512], F32, tag="spp", name="sp")
                    for jj in range(2):
                        kb = 2 * kp + jj
                        off = max(0, kb * 128 - q4 * 512)
                        diag = kb * 128 >= q4 * 512
                        if diag:
                            # preset masked region with NEG (rect for the
                            # pair-mate gap of odd blocks, triangle at off),
                            # then accumulate S on top of it; cols beyond the
                            # preset get their own start=True matmul
                            p0 = po if jj == 1 else off
                            if off + 128 < 512:
                                nc.tensor.matmul(
                                    sp[:, jj, off + 128:512],
                                    kt_slice(h, kb),
                                    q_slice(h, q4, off + 128, 512),
                                    start=True, stop=True,
                                )
                            nc.tensor.matmul(
                                sp[:, jj, p0:off + 128],
                                idn[:],
                                maskc[:, 128 - (off - p0):256],
                                start=True, stop=False,
                            )
                            nc.tensor.matmul(
                                sp[:, jj, off:off + 128],
                                kt_slice(h, kb),
                                q_slice(h, q4, off, off + 128),
                                start=False, stop=True,
                            )
                        else:
                            nc.tensor.matmul(
                                sp[:, jj, 0:512],
                                kt_slice(h, kb),
                                q_slice(h, q4, 0, 512),
                                start=True, stop=True,
                            )
                    if q4 == 0:
                        # bf16 z + per-block bf16 AV (small-q rows average
                        # over too few tokens to tolerate fp8 noise)
                        zt = zpoolb.tile([128, 2, 512], BF16, tag="ztb",
                                         name="ztb")
                        nc.scalar.activation(
                            zt[:, :, po:512], sp[:, :, po:512],
                            mybir.ActivationFunctionType.Exp,
                            scale=float(EXPS),
                        )

                        def av(zt=zt, kp=kp, h=h):
                            for jj in range(2):
                                kb = 2 * kp + jj
                                off = kb * 128
                                nc.tensor.matmul(
                                    yzs[h][:, off:512],
                                    vpb[kb][:, h * EP:(h + 1) * EP],
                                    zt[:, jj, off:512],
                                    start=(kb == 0), stop=(kb == 3),
                                )
                    else:
                        zt = zpool.tile([128, 2, 512], FP8, tag="zt", name="zt")
                        nc.scalar.activation(
                            zt[:, :, po:512], sp[:, :, po:512],
                            mybir.ActivationFunctionType.Exp,
                            scale=float(EXPS),
                        )

                        def av(zt=zt, kp=kp, h=h, po=po):
                            nc.tensor.matmul(
                                yzs[h][:, po:512],
                                vp[kp][:, :, h * EP:(h + 1) * EP],
                                zt[:, :, po:512],
                                start=(kp == 0), stop=(kp == nkp - 1),
                                perf_mode=DR,
                            )
                    # deferred AV: emit the PREVIOUS group's AV so its exp has
                    # had a full group's worth of PE time to complete
                    if prev_av is not None:
                        prev_av()
                    prev_av = av
                    if fill is None or not fill.tick():
                        # keep the HAM activity monitor warm: PE must stay
                        # saturated or it drops to 1.2GHz and sticks there
                        dummy(512 if q4 == 3 else 256)
            if prev_av is not None:
                prev_av()
            dens, rcs, bcs = {}, {}, {}
            for h in heads:
                dens[h] = spool.tile([1, 512], F32, tag="den0", name="den0")
                nc.vector.tensor_copy(dens[h][:], yzs[h][64:65, :])
            dummy()
            for h in heads:
                rcs[h] = spool.tile([1, 512], F32, tag="rc", name="rc")
                nc.vector.reciprocal_approx_fast(rcs[h][:], dens[h][:])
            for h in heads:
                bcs[h] = spool.tile([64, 512], F32, tag="bc_sb", name="bc_sb")
                nc.gpsimd.partition_broadcast(bcs[h][:], rcs[h][:])
            dummy()
            for h in heads:
                nc.vector.tensor_mul(
                    yts[h // 2][(h % 2) * 64:(h % 2) * 64 + 64, :],
                    yzs[h][0:64, :], bcs[h][:],
                )

        def proj_unit(q4, yts, qt):
            ot = opool.tile([128, C], BF16, tag="ot", name="ot")
            for half in range(2):
                pp = pvpp.tile([128, 512], F32, tag="pvpp", name="pp")
                for hdc in range(FV // 128):
                    nc.tensor.matmul(
                        pp[:, 0:384],
                        yts[hdc][:, qt * 128:(qt + 1) * 128],
                        wp_sb[hdc][:, half * 384:(half + 1) * 384],
                        start=(hdc == 0), stop=(hdc == FV // 128 - 1),
                    )
                nc.vector.tensor_copy(
                    ot[:, half * 384:(half + 1) * 384], pp[:, 0:384])
            row = (q4 * 4 + qt) * 128
            nc.sync.dma_start(yp[row:row + 128, :], ot[:])

        class Fill:
            """Evenly spreads independent PE work units across the attention
            groups so the tensor engine never idles waiting on exp."""

            def __init__(self, units, total_groups):
                self.units = list(units)
                self.n0 = len(self.units)
                self.total = max(1, total_groups)
                self.seen = 0

            def tick(self):
                self.seen += 1
                want = self.n0 * self.seen // self.total
                done = self.n0 - len(self.units)
                emitted = False
                while done < want and self.units:
                    self.units.pop(0)()
                    done += 1
                    emitted = True
                return emitted

            def drain(self):
                while self.units:
                    self.units.pop(0)()

        def cc_unit(t4, j):
            tt = t4 * 4 + j
            pv = pvpp.tile([128, HPC * EP], F32, tag="pvpp", name="pv")
            for cp in range(CP):
                nc.tensor.matmul(
                    pv[:],
                    xT[cp][:, :, tt * 128:(tt + 1) * 128],
                    wv_sb[cp][:, :, 0:HPC * EP],
                    start=(cp == 0),
                    stop=(cp == CP - 1),
                    perf_mode=DR,
                )
            nc.vector.tensor_add(
                vp[tt // 2][:, tt % 2, 0:HPC * EP], pv[:], bvb_sb[:])

        # prologue: x chunk 0, Q/K for chunk 0, bf16 V' (needed by q4=0 attn)
        a_chunk(0)
        for r in range(3):
            b_round(0, r)
        ccb_unit(0)
        ccb_unit(1)
        pending = None
        for q4 in range(QC):
            if q4 + 1 < QC:
                a_chunk(q4 + 1)
            yts = [ypool.tile([128, 512], BF16, tag=f"yt{i}", name=f"yt{i}")
                   for i in range(3)]
            units = []
            if pending is not None:
                pq4, pyts = pending
                units += [
                    (lambda qt=qt, pq4=pq4, pyts=pyts: proj_unit(pq4, pyts, qt))
                    for qt in range(4)]
            if q4 == 0:
                units += [(lambda tt=tt: ccb_unit(tt)) for tt in (2, 3)]
                units += [(lambda j=j: cc_unit(0, j)) for j in range(4)]
            if q4 + 1 < QC:
                br = [(lambda r=r: b_round(q4 + 1, r)) for r in range(3)]
                cc = [(lambda j=j: cc_unit(q4 + 1, j)) for j in range(4)]
                # round-robin the unit kinds
                mix = []
                while br or cc:
                    if cc:
                        mix.append(cc.pop(0))
                    if br:
                        mix.append(br.pop(0))
                units += mix
            fill = Fill(units, (2 * q4 + 2) * 6)
            attn_heads(q4, yts, [0, 1], fill)
            attn_heads(q4, yts, [2, 3], fill)
            attn_heads(q4, yts, [4, 5], fill)
            fill.drain()
            if dbg is not None and q4 == 0:
                nc.sync.dma_start(dbg["dqkt0"][:, 0:512], qkt[0][:, 0:512])
                nc.sync.dma_start(dbg["dqkt3"][:, 0:512], qkt[3][:, 0:512])
                nc.sync.dma_start(
                    dbg["dvp0"][:, 0:HPC * EP], vp[0][:, 0, 0:HPC * EP])
                nc.sync.dma_start(
                    dbg["dvp0"][:, 512:512 + HPC * EP], vp[0][:, 1, 0:HPC * EP])
                nc.sync.dma_start(dbg["dyts0"][:], yts[0][:])
                nc.sync.dma_start(dbg["dyts1"][:], yts[1][:])
            pending = (q4, yts)
        for qt in range(4):
            proj_unit(pending[0], pending[1], qt)


_PROGRAM = None


def _get_program():
    global _PROGRAM
    if _PROGRAM is None:
        _PROGRAM = _build_program()
    return _PROGRAM


def _pack_pairs(w):
    """[C, F] -> [CP*128, 2*F]: row r=cp*128+p, col j*F+f = w[(2cp+j)*128+p, f]."""
    F = w.shape[1]
    out = np.zeros((CP * 128, 2 * F), dtype=w.dtype)
    for cp in range(CP):
        for j in range(2):
            out[cp * 128:(cp + 1) * 128, j * F:(j + 1) * F] = \
                w[(2 * cp + j) * 128:(2 * cp + j + 1) * 128, :]
    return out


def _pad_wv(wv):
    out = np.zeros((C, HPC * EP), dtype=np.float32)
    for h in range(HPC):
        out[:, h * EP:h * EP + D] = wv[:, h * D:(h + 1) * D]
    return out


def kernel(x, W_attn, b_attn, W_proj, b_proj):
    x = np.ascontiguousarray(x, dtype=np.float32)
    W_attn = np.ascontiguousarray(W_attn, dtype=np.float32)
    b_attn = np.ascontiguousarray(b_attn, dtype=np.float32)
    W_proj = np.ascontiguousarray(W_proj, dtype=np.float32)
    b_proj = np.ascontiguousarray(b_proj, dtype=np.float32)

    nc = _get_program()
    mask_const = np.ones((128, 256), np.float32)
    mask_const[:, 128:] = np.tril(np.ones((128, 128), np.float32), -1)
    mask_const = mask_const.astype(NPBF)
    idn_const = (NEG * np.eye(128, dtype=np.float32)).astype(NPBF)

    in_maps = []
    for core in range(N_CORES):
        b, g = core // 2, core % 2
        qcols = slice(384 * g, 384 * (g + 1))
        kcols = slice(768 + 384 * g, 768 + 384 * (g + 1))
        vcols = slice(1536 + 384 * g, 1536 + 384 * (g + 1))
        wqk_full = np.concatenate(
            [W_attn[:, qcols], W_attn[:, kcols]], axis=1) * WS
        wv_full = _pad_wv(W_attn[:, vcols]) * WS
        bvb_row = np.zeros((HPC * EP,), np.float32)
        for h in range(HPC):
            bvb_row[h * EP:h * EP + D] = WS * b_attn[vcols][h * D:(h + 1) * D]
            bvb_row[h * EP + D] = WS
        xt = np.ascontiguousarray(x[b].T)
        in_maps.append({
            "xtd": xt.astype(NPF8),
            "xbd": np.ascontiguousarray(xt[:, :512]).astype(NPBF),
            "wqk": _pack_pairs(wqk_full.astype(NPF8)),
            "wv": _pack_pairs(wv_full.astype(NPF8)),
            "wvb": wv_full.astype(NPBF),
            "bqk": np.ascontiguousarray(
                WS * np.concatenate([b_attn[qcols], b_attn[kcols]])),
            "bvb": np.ascontiguousarray(
                np.broadcast_to(bvb_row, (128, HPC * EP))),
            "wp": np.ascontiguousarray(
                W_proj[384 * g:384 * (g + 1), :]).astype(NPBF),
            "maskd": mask_const,
            "idnd": idn_const,
        })

    trace = bool(int(os.environ.get("KBENCH_TRACE", "0")))
    if trace:
        _install_ntff_hook()
    res = run_bass_kernel_spmd(
        nc, in_maps, list(range(N_CORES)), trace=trace,
    )
    kernel.last_exec_time_ns = res.exec_time_ns

    out = np.empty((B, T, C), dtype=np.float32)
    for b in range(B):
        out[b] = (res.results[2 * b]["yp"].astype(np.float32)
                  + res.results[2 * b + 1]["yp"].astype(np.float32) + b_proj)
    return out


# revision 31
# speedup vs baseline: 1.0136x; 1.0136x over previous
"""Causal self-attention (B=4, T=2048, C=768, H=12) on 8 trn2 NeuronCores.

Sharding: 8 cores = 4 batches x 2 head-groups (6 heads each).
Each core: QKV projection for its 6 heads, causal attention, partial output
projection (row-parallel). Host sums the two partials per batch + b_proj.

Device-side dataflow (v2, fp8 DoubleRow):
  - x shipped as x^T fp8e4 (values ~N(0,1), well inside e4m3 range).
  - QKV weights host-scaled by 32 (avoids fp8 subnormals) and packed in
    contraction PAIRS for DoubleRow matmuls: lhsT/rhs APs [128, 2, M].
    Q,K come out x32; the 1/32^2 is folded into the exp scale (1/8192).
    V comes out x32 with a x32 ones column, so y = (32*sum z v)/(32*sum z)
    is exactly unscaled.
  - Causal mask: PE "preset" matmuls accumulate -245760 * [ones|L] into the
    masked PSUM region before the S matmul lands; exp(x/8192) then flushes
    those entries to exactly 0 in fp8. No vector-engine masking at all.
  - S^T computed per k-block into [128, 2, 512] PSUM pair tiles (2 banks);
    ONE exp activation per (k-pair, head) writes the fp8 z pair tile.
  - AV matmul in fp8 DoubleRow over k-block pairs; the V' ones column
    yields the softmax denominator for free. y normalized via
    reciprocal_approx_fast + gpsimd partition_broadcast + vector multiply.
  - Output projection in bf16, result DMA'd out as bf16 partials.
"""

import os
import sys
import types

sys.path.insert(0, "/opt/trn_rl_repo")

import ml_dtypes
import numpy as np

import concourse.bass as bass
import concourse.tile as tile
from concourse import bacc, mybir
from concourse.bass_utils import run_bass_kernel_spmd

B, T, C, H, D = 4, 2048, 768, 12, 64
N_CORES = 8
HPC = H // 2          # heads per core = 6
FQK = 2 * HPC * D     # 768 qk features per core
FV = HPC * D          # 384 v features per core
E = D + 1             # 65: head dim + ones column
EP = 68               # padded V' per-head width (dual-fp8 ldweights alignment)
CCH = C // 128        # 6 contraction chunks
CP = CCH // 2         # 3 contraction chunk pairs (DoubleRow)
QC = T // 512         # 4 query chunks of 512
F32 = mybir.dt.float32
BF16 = mybir.dt.bfloat16
FP8 = mybir.dt.float8e4
NPBF = ml_dtypes.bfloat16
NPF8 = ml_dtypes.float8_e4m3
DR = mybir.MatmulPerfMode.DoubleRow

WS = 32.0             # host weight scale (fp8 subnormal avoidance)
EXPS = 1.0 / (np.sqrt(D) * WS * WS)   # 1/(8*1024) = 1/8192
NEG = -30.0 * WS * WS * np.sqrt(D)    # -245760; exp(NEG*EXPS) == exp(-30)


def _install_ntff_hook():
    """The image's antenv lacks axon_hooks; inject it so trace=True works."""
    if "antenv.axon_hooks" in sys.modules:
        return
    try:
        import antenv
        mod = types.ModuleType("antenv.axon_hooks")
        _state = {"hook": None}
        mod.set_axon_ntff_profile_hook = lambda h: _state.__setitem__("hook", h)
        mod.get_axon_ntff_profile_hook = lambda: _state["hook"]
        sys.modules["antenv.axon_hooks"] = mod
        antenv.axon_hooks = mod
        from trn_agent_boot.trn_boot import _ntff_profile_via_ctypes
        mod.set_axon_ntff_profile_hook(
            _ntff_profile_via_ctypes("/opt/axon/libaxon_pjrt.so")
        )
    except Exception:
        pass


def _build_program():
    nc = bacc.Bacc(
        "TRN2",
        target_bir_lowering=False,
        debug=False,
        enable_asserts=False,
        num_devices=N_CORES,
    )
    xtd = nc.dram_tensor("xtd", [C, T], FP8, kind="ExternalInput").ap()
    xbd = nc.dram_tensor("xbd", [C, 512], BF16, kind="ExternalInput").ap()
    wqk = nc.dram_tensor("wqk", [CP * 128, 2 * FQK], FP8, kind="ExternalInput").ap()
    wv = nc.dram_tensor("wv", [CP * 128, 2 * HPC * EP], FP8, kind="ExternalInput").ap()
    wvb = nc.dram_tensor("wvb", [C, HPC * EP], BF16, kind="ExternalInput").ap()
    bqk = nc.dram_tensor("bqk", [FQK], F32, kind="ExternalInput").ap()
    bvb = nc.dram_tensor("bvb", [128, HPC * EP], F32, kind="ExternalInput").ap()
    wp = nc.dram_tensor("wp", [FV, C], BF16, kind="ExternalInput").ap()
    maskd = nc.dram_tensor("maskd", [128, 256], BF16, kind="ExternalInput").ap()
    idnd = nc.dram_tensor("idnd", [128, 128], BF16, kind="ExternalInput").ap()
    yp = nc.dram_tensor("yp", [T, C], BF16, kind="ExternalOutput").ap()
    dbg = None
    if os.environ.get("KDEBUG"):
        dbg = {
            "dqkt0": nc.dram_tensor("dqkt0", [128, T], BF16,
                                    kind="ExternalOutput").ap(),
            "dqkt3": nc.dram_tensor("dqkt3", [128, T], BF16,
                                    kind="ExternalOutput").ap(),
            "dvp0": nc.dram_tensor("dvp0", [128, 1024], FP8,
                                   kind="ExternalOutput").ap(),
            "dyts0": nc.dram_tensor("dyts0", [128, 512], BF16,
                                    kind="ExternalOutput").ap(),
            "dyts1": nc.dram_tensor("dyts1", [128, 512], BF16,
                                    kind="ExternalOutput").ap(),
            "dsp": nc.dram_tensor("dsp", [128, 1024], F32,
                                  kind="ExternalOutput").ap(),
            "dzt": nc.dram_tensor("dzt", [128, 1024], BF16,
                                  kind="ExternalOutput").ap(),
            "dyz": nc.dram_tensor("dyz", [EP, 512], F32,
                                  kind="ExternalOutput").ap(),
            "drc": nc.dram_tensor("drc", [1, 512], F32,
                                  kind="ExternalOutput").ap(),
        }

    with tile.TileContext(nc) as tc:
        _body(tc, nc, xtd, xbd, wqk, wv, wvb, bqk, bvb, wp, maskd, idnd, yp,
              dbg)

    nc.compile()
    return nc


def _body(tc, nc, xtd, xbd, wqk, wv, wvb, bqk, bvb, wp, maskd, idnd, yp,
          dbg=None):
    from contextlib import ExitStack

    with ExitStack() as es:
        persist = es.enter_context(tc.tile_pool(name="persist", bufs=1))
        # S pair tiles + QKV-proj pair tiles share one 2-bank-tile pool
        spp = es.enter_context(tc.tile_pool(name="spp", bufs=2, space="PSUM"))
        pvpp = es.enter_context(tc.tile_pool(name="pvpp", bufs=2, space="PSUM"))
        psyz = es.enter_context(tc.tile_pool(name="psyz", bufs=2, space="PSUM"))
        zpool = es.enter_context(tc.tile_pool(name="zpool", bufs=5))
        zpoolb = es.enter_context(tc.tile_pool(name="zpoolb", bufs=3))
        ypool = es.enter_context(tc.tile_pool(name="ypool", bufs=2))
        opool = es.enter_context(tc.tile_pool(name="opool", bufs=3))
        spool = es.enter_context(tc.tile_pool(name="spool", bufs=4))

        # ---- small constants first (scalar queue) so big loads start early
        maskc = persist.tile([128, 256], BF16, tag="maskc", name="maskc")
        nc.scalar.dma_start(maskc[:], maskd[:])
        idn = persist.tile([128, 128], BF16, tag="idn", name="idn")
        nc.scalar.dma_start(idn[:], idnd[:])
        bqk_sb = persist.tile([128, CCH], F32, tag="bqk", name="bqk_sb")
        nc.scalar.dma_start(bqk_sb[:], bqk.rearrange("(f p) -> p f", p=128))
        bvb_sb = persist.tile([128, HPC * EP], F32, tag="bvb", name="bvb_sb")
        nc.scalar.dma_start(bvb_sb[:], bvb[:])

        wqk_sb = [persist.tile([128, 2, FQK], FP8, tag=f"wqk{i}", name=f"wqk_sb{i}")
                  for i in range(CP)]
        # j-stride padded to 512 (dual-fp8 ISA: k-tile stride must be
        # 16-byte aligned)
        wv_sb = [persist.tile([128, 2, 512], FP8, tag=f"wv{i}", name=f"wv_sb{i}")
                 for i in range(CP)]
        wp_sb = [persist.tile([128, C], BF16, tag=f"wp{i}", name=f"wp_sb{i}")
                 for i in range(FV // 128)]
        for i in range(CP):
            for j in range(2):
                nc.gpsimd.dma_start(
                    wqk_sb[i][:, j, :],
                    wqk[i * 128:(i + 1) * 128, j * FQK:(j + 1) * FQK])
        for i in range(CP):
            for j in range(2):
                nc.gpsimd.dma_start(
                    wv_sb[i][:, j, 0:HPC * EP],
                    wv[i * 128:(i + 1) * 128, j * HPC * EP:(j + 1) * HPC * EP])
        for i in range(FV // 128):
            nc.gpsimd.dma_start(wp_sb[i][:], wp[i * 128:(i + 1) * 128, :])

        # x^T fp8, stored as chunk-pair tiles for DoubleRow
        xT = [persist.tile([128, 2, T], FP8, tag=f"xT{i}", name=f"xT{i}")
              for i in range(CP)]
        # QK^T bf16: tiles 0..2 hold Q^T (6 heads x 64), 3..5 hold K^T
        qkt = [persist.tile([128, T], BF16, tag=f"qkt{i}", name=f"qkt{i}")
               for i in range(CCH)]
        # V' fp8 pair tiles: [tok 128, kpair j, 6*65 feats] per k-block pair
        vp = [persist.tile([128, 2, 512], FP8, tag=f"vp{i}", name=f"vp{i}")
              for i in range(T // 256)]
        # bf16 V' for the first 4 token blocks (used by the q4=0 bf16 path)
        vpb = [persist.tile([128, HPC * EP], BF16, tag=f"vpb{i}", name=f"vpb{i}")
               for i in range(4)]
        # bf16 x^T (first 512 tokens) + bf16 wv for the q4=0 V projection
        xTb = [persist.tile([128, 512], BF16, tag=f"xTb{i}", name=f"xTb{i}")
               for i in range(CCH)]
        wvb_sb = [persist.tile([128, HPC * EP], BF16, tag=f"wvb{i}",
                               name=f"wvb_sb{i}") for i in range(CCH)]
        for i in range(CCH):
            nc.gpsimd.dma_start(wvb_sb[i][:], wvb[i * 128:(i + 1) * 128, :])
        for i in range(CCH):
            nc.gpsimd.dma_start(xTb[i][:], xbd[i * 128:(i + 1) * 128, :])

        def a_chunk(t4):
            # DMA this chunk's x^T columns (pre-transposed fp8 on host)
            for cp in range(CP):
                for j in range(2):
                    nc.sync.dma_start(
                        xT[cp][:, j, t4 * 512:(t4 + 1) * 512],
                        xtd[(2 * cp + j) * 128:(2 * cp + j + 1) * 128,
                            t4 * 512:(t4 + 1) * 512],
                    )

        def b_round(q4, r):
            # Q^T (ft=r) and K^T (ft=r+3) for head pair 2r, 2r+1
            for ft in (r, r + 3):
                ps = pvpp.tile([128, 512], F32, tag="pvpp", name="ps",
                               padded_shape=[128, 512])
                for cp in range(CP):
                    nc.tensor.matmul(
                        ps[:],
                        wqk_sb[cp][:, :, ft * 128:(ft + 1) * 128],
                        xT[cp][:, :, q4 * 512:(q4 + 1) * 512],
                        start=(cp == 0),
                        stop=(cp == CP - 1),
                        perf_mode=DR,
                    )
                nc.vector.tensor_scalar_add(
                    qkt[ft][:, q4 * 512:(q4 + 1) * 512],
                    ps[:],
                    bqk_sb[:, ft:ft + 1],
                )

        def c_chunk(t4):
            # V' fp8 pair tiles for 4 token blocks
            for j in range(4):
                tt = t4 * 4 + j
                pv = pvpp.tile([128, HPC * EP], F32, tag="pvpp", name="pv")
                for cp in range(CP):
                    nc.tensor.matmul(
                        pv[:],
                        xT[cp][:, :, tt * 128:(tt + 1) * 128],
                        wv_sb[cp][:, :, 0:HPC * EP],
                        start=(cp == 0),
                        stop=(cp == CP - 1),
                        perf_mode=DR,
                    )
                nc.vector.tensor_add(
                    vp[tt // 2][:, tt % 2, 0:HPC * EP], pv[:], bvb_sb[:])

        def ccb_unit(tt):
            # bf16 V' for one of token blocks 0..3 (q4=0 bf16 AV path)
            pv = pvpp.tile([128, HPC * EP], F32, tag="pvpp", name="pvb")
            for cc in range(CCH):
                nc.tensor.matmul(
                    pv[:],
                    xTb[cc][:, tt * 128:(tt + 1) * 128],
                    wvb_sb[cc][:],
                    start=(cc == 0),
                    stop=(cc == CCH - 1),
                )
            nc.vector.tensor_add(vpb[tt][:], pv[:], bvb_sb[:])

        def kt_slice(h, kb):
            return qkt[3 + h // 2][(h % 2) * 64:(h % 2) * 64 + 64,
                                   kb * 128:(kb + 1) * 128]

        def q_slice(h, q4, c0, c1):
            return qkt[h // 2][(h % 2) * 64:(h % 2) * 64 + 64,
                               q4 * 512 + c0:q4 * 512 + c1]

        def dummy(n=256):
            pv = pvpp.tile([128, 512], F32, tag="pvpp", name="dum")
            nc.tensor.matmul(pv[:, 0:n], idn[:], xTb[0][:, 0:n],
                             start=True, stop=True)

        def attn_heads(q4, yts, heads, fill=None):
            nkp = 2 * q4 + 2
            yzs = {h: psyz.tile([EP, 512], F32, tag="yz", name=f"yz{h}")
                   for h in heads}
            prev_av = None
            for kp in range(nkp):
                po = max(0, kp * 256 - q4 * 512)  # pair col offset
                for h in heads:
                    sp = spp.tile([128, 2, 512], F32, tag="spp", name="sp")
                    for jj in range(2):
                        kb = 2 * kp + jj
                        off = max(0, kb * 128 - q4 * 512)
                        diag = kb * 128 >= q4 * 512
                        if diag:
                            # preset masked region with NEG (rect for the
                            # pair-mate gap of odd blocks, triangle at off),
                            # then accumulate S on top of it; cols beyond the
                            # preset get their own start=True matmul
                            p0 = po if jj == 1 else off
                            nc.tensor.matmul(
                                sp[:, jj, p0:off + 128],
                                idn[:],
                                maskc[:, 128 - (off - p0):256],
                                start=True, stop=False,
                            )
                            nc.tensor.matmul(
                                sp[:, jj, off:off + 128],
                                kt_slice(h, kb),
                                q_slice(h, q4, off, off + 128),
                                start=False, stop=True,
                            )
                            if off + 128 < 512:
                                nc.tensor.matmul(
                                    sp[:, jj, off + 128:512],
                                    kt_slice(h, kb),
                                    q_slice(h, q4, off + 128, 512),
                                    start=True, stop=True,
                                )
                        else:
                            nc.tensor.matmul(
                                sp[:, jj, 0:512],
                                kt_slice(h, kb),
                                q_slice(h, q4, 0, 512),
                                start=True, stop=True,
                            )
                    if q4 == 0:
                        # bf16 z + per-block bf16 AV (small-q rows average
                        # over too few tokens to tolerate fp8 noise)
                        zt = zpoolb.tile([128, 2, 512], BF16, tag="ztb",
                                         name="ztb")
                        nc.scalar.activation(
                            zt[:, :, po:512], sp[:, :, po:512],
                            mybir.ActivationFunctionType.Exp,
                            scale=float(EXPS),
                        )

                        def av(zt=zt, kp=kp, h=h):
                            for jj in range(2):
                                kb = 2 * kp + jj
                                off = kb * 128
                                nc.tensor.matmul(
                                    yzs[h][:, off:512],
                                    vpb[kb][:, h * EP:(h + 1) * EP],
                                    zt[:, jj, off:512],
                                    start=(kb == 0), stop=(kb == 3),
                                )
                    else:
                        zt = zpool.tile([128, 2, 512], FP8, tag="zt", name="zt")
                        nc.scalar.activation(
                            zt[:, :, po:512], sp[:, :, po:512],
                            mybir.ActivationFunctionType.Exp,
                            scale=float(EXPS),
                        )

                        def av(zt=zt, kp=kp, h=h, po=po):
                            nc.tensor.matmul(
                                yzs[h][:, po:512],
                                vp[kp][:, :, h * EP:(h + 1) * EP],
                                zt[:, :, po:512],
                                start=(kp == 0), stop=(kp == nkp - 1),
                                perf_mode=DR,
                            )
                    # deferred AV: emit the PREVIOUS group's AV so its exp has
                    # had a full group's worth of PE time to complete
                    if prev_av is not None:
                        prev_av()
                    prev_av = av
                    if fill is None or not fill.tick():
                        # keep the HAM activity monitor warm: PE must stay
                        # saturated or it drops to 1.2GHz and sticks there
                        dummy(512 if q4 == 3 else 256)
            if prev_av is not None:
                prev_av()
            dens, rcs, bcs = {}, {}, {}
            for h in heads:
                dens[h] = spool.tile([1, 512], F32, tag="den0", name="den0")
                nc.vector.tensor_copy(dens[h][:], yzs[h][64:65, :])
            dummy()
            for h in heads:
                rcs[h] = spool.tile([1, 512], F32, tag="rc", name="rc")
                nc.vector.reciprocal_approx_fast(rcs[h][:], dens[h][:])
            for h in heads:
                bcs[h] = spool.tile([64, 512], F32, tag="bc_sb", name="bc_sb")
                nc.gpsimd.partition_broadcast(bcs[h][:], rcs[h][:])
            dummy()
            for h in heads:
                nc.vector.tensor_mul(
                    yts[h // 2][(h % 2) * 64:(h % 2) * 64 + 64, :],
                    yzs[h][0:64, :], bcs[h][:],
                )

        def proj_unit(q4, yts, qt):
            ot = opool.tile([128, C], BF16, tag="ot", name="ot")
            for half in range(2):
                pp = pvpp.tile([128, 512], F32, tag="pvpp", name="pp")
                for hdc in range(FV // 128):
                    nc.tensor.matmul(
                        pp[:, 0:384],
                        yts[hdc][:, qt * 128:(qt + 1) * 128],
                        wp_sb[hdc][:, half * 384:(half + 1) * 384],
                        start=(hdc == 0), stop=(hdc == FV // 128 - 1),
                    )
                nc.vector.tensor_copy(
                    ot[:, half * 384:(half + 1) * 384], pp[:, 0:384])
            row = (q4 * 4 + qt) * 128
            nc.sync.dma_start(yp[row:row + 128, :], ot[:])

        class Fill:
            """Evenly spreads independent PE work units across the attention
            groups so the tensor engine never idles waiting on exp."""

            def __init__(self, units, total_groups):
                self.units = list(units)
                self.n0 = len(self.units)
                self.total = max(1, total_groups)
                self.seen = 0

            def tick(self):
                self.seen += 1
                want = self.n0 * self.seen // self.total
                done = self.n0 - len(self.units)
                emitted = False
                while done < want and self.units:
                    self.units.pop(0)()
                    done += 1
                    emitted = True
                return emitted

            def drain(self):
                while self.units:
                    self.units.pop(0)()

        def cc_unit(t4, j):
            tt = t4 * 4 + j
            pv = pvpp.tile([128, HPC * EP], F32, tag="pvpp", name="pv")
            for cp in range(CP):
                nc.tensor.matmul(
                    pv[:],
                    xT[cp][:, :, tt * 128:(tt + 1) * 128],
                    wv_sb[cp][:, :, 0:HPC * EP],
                    start=(cp == 0),
                    stop=(cp == CP - 1),
                    perf_mode=DR,
                )
            nc.vector.tensor_add(
                vp[tt // 2][:, tt % 2, 0:HPC * EP], pv[:], bvb_sb[:])

        # prologue: x chunk 0, Q/K for chunk 0, bf16 V' (needed by q4=0 attn)
        a_chunk(0)
        for r in range(3):
            b_round(0, r)
        ccb_unit(0)
        ccb_unit(1)
        pending = None
        for q4 in range(QC):
            if q4 + 1 < QC:
                a_chunk(q4 + 1)
            yts = [ypool.tile([128, 512], BF16, tag=f"yt{i}", name=f"yt{i}")
                   for i in range(3)]
            units = []
            if pending is not None:
                pq4, pyts = pending
                units += [
                    (lambda qt=qt, pq4=pq4, pyts=pyts: proj_unit(pq4, pyts, qt))
                    for qt in range(4)]
            if q4 == 0:
                units += [(lambda tt=tt: ccb_unit(tt)) for tt in (2, 3)]
                units += [(lambda j=j: cc_unit(0, j)) for j in range(4)]
            if q4 + 1 < QC:
                br = [(lambda r=r: b_round(q4 + 1, r)) for r in range(3)]
                cc = [(lambda j=j: cc_unit(q4 + 1, j)) for j in range(4)]
                # round-robin the unit kinds
                mix = []
                while br or cc:
                    if cc:
                        mix.append(cc.pop(0))
                    if br:
                        mix.append(br.pop(0))
                units += mix
            fill = Fill(units, (2 * q4 + 2) * 6)
            attn_heads(q4, yts, [0, 1], fill)
            attn_heads(q4, yts, [2, 3], fill)
            attn_heads(q4, yts, [4, 5], fill)
            fill.drain()
            if dbg is not None and q4 == 0:
                nc.sync.dma_start(dbg["dqkt0"][:, 0:512], qkt[0][:, 0:512])
                nc.sync.dma_start(dbg["dqkt3"][:, 0:512], qkt[3][:, 0:512])
                nc.sync.dma_start(
                    dbg["dvp0"][:, 0:HPC * EP], vp[0][:, 0, 0:HPC * EP])
                nc.sync.dma_start(
                    dbg["dvp0"][:, 512:512 + HPC * EP], vp[0][:, 1, 0:HPC * EP])
                nc.sync.dma_start(dbg["dyts0"][:], yts[0][:])
                nc.sync.dma_start(dbg["dyts1"][:], yts[1][:])
            pending = (q4, yts)
        for qt in range(4):
            proj_unit(pending[0], pending[1], qt)


_PROGRAM = None


def _get_program():
    global _PROGRAM
    if _PROGRAM is None:
        _PROGRAM = _build_program()
    return _PROGRAM


def _pack_pairs(w):
    """[C, F] -> [CP*128, 2*F]: row r=cp*128+p, col j*F+f = w[(2cp+j)*128+p, f]."""
    F = w.shape[1]
    out = np.zeros((CP * 128, 2 * F), dtype=w.dtype)
    for cp in range(CP):
        for j in range(2):
            out[cp * 128:(cp + 1) * 128, j * F:(j + 1) * F] = \
                w[(2 * cp + j) * 128:(2 * cp + j + 1) * 128, :]
    return out


def _pad_wv(wv):
    out = np.zeros((C, HPC * EP), dtype=np.float32)
    for h in range(HPC):
        out[:, h * EP:h * EP + D] = wv[:, h * D:(h + 1) * D]
    return out


def kernel(x, W_attn, b_attn, W_proj, b_proj):
    x = np.ascontiguousarray(x, dtype=np.float32)
    W_attn = np.ascontiguousarray(W_attn, dtype=np.float32)
    b_attn = np.ascontiguousarray(b_attn, dtype=np.float32)
    W_proj = np.ascontiguousarray(W_proj, dtype=np.float32)
    b_proj = np.ascontiguousarray(b_proj, dtype=np.float32)

    nc = _get_program()
    mask_const = np.ones((128, 256), np.float32)
    mask_const[:, 128:] = np.tril(np.ones((128, 128), np.float32), -1)
    mask_const = mask_const.astype(NPBF)
    idn_const = (NEG * np.eye(128, dtype=np.float32)).astype(NPBF)

    in_maps = []
    for core in range(N_CORES):
        b, g = core // 2, core % 2
        qcols = slice(384 * g, 384 * (g + 1))
        kcols = slice(768 + 384 * g, 768 + 384 * (g + 1))
        vcols = slice(1536 + 384 * g, 1536 + 384 * (g + 1))
        wqk_full = np.concatenate(
            [W_attn[:, qcols], W_attn[:, kcols]], axis=1) * WS
        wv_full = _pad_wv(W_attn[:, vcols]) * WS
        bvb_row = np.zeros((HPC * EP,), np.float32)
        for h in range(HPC):
            bvb_row[h * EP:h * EP + D] = WS * b_attn[vcols][h * D:(h + 1) * D]
            bvb_row[h * EP + D] = WS
        xt = np.ascontiguousarray(x[b].T)
        in_maps.append({
            "xtd": xt.astype(NPF8),
            "xbd": np.ascontiguousarray(xt[:, :512]).astype(NPBF),
            "wqk": _pack_pairs(wqk_full.astype(NPF8)),
            "wv": _pack_pairs(wv_full.astype(NPF8)),
            "wvb": wv_full.astype(NPBF),
            "bqk": np.ascontiguousarray(
                WS * np.concatenate([b_attn[qcols], b_attn[kcols]])),
            "bvb": np.ascontiguousarray(
                np.broadcast_to(bvb_row, (128, HPC * EP))),
            "wp": np.ascontiguousarray(
                W_proj[384 * g:384 * (g + 1), :]).astype(NPBF),
            "maskd": mask_const,
            "idnd": idn_const,
        })

    trace = bool(int(os.environ.get("KBENCH_TRACE", "0")))
    if trace:
        _install_ntff_hook()
    res = run_bass_kernel_spmd(
        nc, in_maps, list(range(N_CORES)), trace=trace,
    )
    kernel.last_exec_time_ns = res.exec_time_ns

    out = np.empty((B, T, C), dtype=np.float32)
    for b in range(B):
        out[b] = (res.results[2 * b]["yp"].astype(np.float32)
                  + res.results[2 * b + 1]["yp"].astype(np.float32) + b_proj)
    return out


# revision 32
# speedup vs baseline: 1.0249x; 1.0112x over previous
"""Causal self-attention (B=4, T=2048, C=768, H=12) on 8 trn2 NeuronCores.

Sharding: 8 cores = 4 batches x 2 head-groups (6 heads each).
Each core: QKV projection for its 6 heads, causal attention, partial output
projection (row-parallel). Host sums the two partials per batch + b_proj.

Device-side dataflow (v2, fp8 DoubleRow):
  - x shipped as x^T fp8e4 (values ~N(0,1), well inside e4m3 range).
  - QKV weights host-scaled by 32 (avoids fp8 subnormals) and packed in
    contraction PAIRS for DoubleRow matmuls: lhsT/rhs APs [128, 2, M].
    Q,K come out x32; the 1/32^2 is folded into the exp scale (1/8192).
    V comes out x32 with a x32 ones column, so y = (32*sum z v)/(32*sum z)
    is exactly unscaled.
  - Causal mask: PE "preset" matmuls accumulate -245760 * [ones|L] into the
    masked PSUM region before the S matmul lands; exp(x/8192) then flushes
    those entries to exactly 0 in fp8. No vector-engine masking at all.
  - S^T computed per k-block into [128, 2, 512] PSUM pair tiles (2 banks);
    ONE exp activation per (k-pair, head) writes the fp8 z pair tile.
  - AV matmul in fp8 DoubleRow over k-block pairs; the V' ones column
    yields the softmax denominator for free. y normalized via
    reciprocal_approx_fast + gpsimd partition_broadcast + vector multiply.
  - Output projection in bf16, result DMA'd out as bf16 partials.
"""

import os
import sys
import types

sys.path.insert(0, "/opt/trn_rl_repo")

import ml_dtypes
import numpy as np

import concourse.bass as bass
import concourse.tile as tile
from concourse import bacc, mybir
from concourse.bass_utils import run_bass_kernel_spmd

B, T, C, H, D = 4, 2048, 768, 12, 64
N_CORES = 8
HPC = H // 2          # heads per core = 6
FQK = 2 * HPC * D     # 768 qk features per core
FV = HPC * D          # 384 v features per core
E = D + 1             # 65: head dim + ones column
EP = 68               # padded V' per-head width (dual-fp8 ldweights alignment)
CCH = C // 128        # 6 contraction chunks
CP = CCH // 2         # 3 contraction chunk pairs (DoubleRow)
QC = T // 512         # 4 query chunks of 512
F32 = mybir.dt.float32
BF16 = mybir.dt.bfloat16
FP8 = mybir.dt.float8e4
NPBF = ml_dtypes.bfloat16
NPF8 = ml_dtypes.float8_e4m3
DR = mybir.MatmulPerfMode.DoubleRow

WS = 32.0             # host weight scale (fp8 subnormal avoidance)
EXPS = 1.0 / (np.sqrt(D) * WS * WS)   # 1/(8*1024) = 1/8192
NEG = -30.0 * WS * WS * np.sqrt(D)    # -245760; exp(NEG*EXPS) == exp(-30)


def _install_ntff_hook():
    """The image's antenv lacks axon_hooks; inject it so trace=True works."""
    if "antenv.axon_hooks" in sys.modules:
        return
    try:
        import antenv
        mod = types.ModuleType("antenv.axon_hooks")
        _state = {"hook": None}
        mod.set_axon_ntff_profile_hook = lambda h: _state.__setitem__("hook", h)
        mod.get_axon_ntff_profile_hook = lambda: _state["hook"]
        sys.modules["antenv.axon_hooks"] = mod
        antenv.axon_hooks = mod
        from trn_agent_boot.trn_boot import _ntff_profile_via_ctypes
        mod.set_axon_ntff_profile_hook(
            _ntff_profile_via_ctypes("/opt/axon/libaxon_pjrt.so")
        )
    except Exception:
        pass


def _build_program():
    nc = bacc.Bacc(
        "TRN2",
        target_bir_lowering=False,
        debug=False,
        enable_asserts=False,
        num_devices=N_CORES,
    )
    xtd = nc.dram_tensor("xtd", [C, T], FP8, kind="ExternalInput").ap()
    xbd = nc.dram_tensor("xbd", [C, 512], BF16, kind="ExternalInput").ap()
    wqk = nc.dram_tensor("wqk", [CP * 128, 2 * FQK], FP8, kind="ExternalInput").ap()
    wv = nc.dram_tensor("wv", [CP * 128, 2 * HPC * EP], FP8, kind="ExternalInput").ap()
    wvb = nc.dram_tensor("wvb", [C, HPC * EP], BF16, kind="ExternalInput").ap()
    bqk = nc.dram_tensor("bqk", [FQK], F32, kind="ExternalInput").ap()
    bvb = nc.dram_tensor("bvb", [128, HPC * EP], F32, kind="ExternalInput").ap()
    wp = nc.dram_tensor("wp", [FV, C], BF16, kind="ExternalInput").ap()
    maskd = nc.dram_tensor("maskd", [128, 256], BF16, kind="ExternalInput").ap()
    idnd = nc.dram_tensor("idnd", [128, 128], BF16, kind="ExternalInput").ap()
    yp = nc.dram_tensor("yp", [T, C], BF16, kind="ExternalOutput").ap()
    dbg = None
    if os.environ.get("KDEBUG"):
        dbg = {
            "dqkt0": nc.dram_tensor("dqkt0", [128, T], BF16,
                                    kind="ExternalOutput").ap(),
            "dqkt3": nc.dram_tensor("dqkt3", [128, T], BF16,
                                    kind="ExternalOutput").ap(),
            "dvp0": nc.dram_tensor("dvp0", [128, 1024], FP8,
                                   kind="ExternalOutput").ap(),
            "dyts0": nc.dram_tensor("dyts0", [128, 512], BF16,
                                    kind="ExternalOutput").ap(),
            "dyts1": nc.dram_tensor("dyts1", [128, 512], BF16,
                                    kind="ExternalOutput").ap(),
            "dsp": nc.dram_tensor("dsp", [128, 1024], F32,
                                  kind="ExternalOutput").ap(),
            "dzt": nc.dram_tensor("dzt", [128, 1024], BF16,
                                  kind="ExternalOutput").ap(),
            "dyz": nc.dram_tensor("dyz", [EP, 512], F32,
                                  kind="ExternalOutput").ap(),
            "drc": nc.dram_tensor("drc", [1, 512], F32,
                                  kind="ExternalOutput").ap(),
        }

    with tile.TileContext(nc) as tc:
        _body(tc, nc, xtd, xbd, wqk, wv, wvb, bqk, bvb, wp, maskd, idnd, yp,
              dbg)

    nc.compile()
    return nc


def _body(tc, nc, xtd, xbd, wqk, wv, wvb, bqk, bvb, wp, maskd, idnd, yp,
          dbg=None):
    from contextlib import ExitStack

    with ExitStack() as es:
        persist = es.enter_context(tc.tile_pool(name="persist", bufs=1))
        # S pair tiles + QKV-proj pair tiles share one 2-bank-tile pool
        spp = es.enter_context(tc.tile_pool(name="spp", bufs=2, space="PSUM"))
        pvpp = es.enter_context(tc.tile_pool(name="pvpp", bufs=2, space="PSUM"))
        psyz = es.enter_context(tc.tile_pool(name="psyz", bufs=2, space="PSUM"))
        zpool = es.enter_context(tc.tile_pool(name="zpool", bufs=5))
        zpoolb = es.enter_context(tc.tile_pool(name="zpoolb", bufs=3))
        ypool = es.enter_context(tc.tile_pool(name="ypool", bufs=2))
        opool = es.enter_context(tc.tile_pool(name="opool", bufs=3))
        spool = es.enter_context(tc.tile_pool(name="spool", bufs=4))

        # ---- small constants first (scalar queue) so big loads start early
        maskc = persist.tile([128, 256], BF16, tag="maskc", name="maskc")
        nc.scalar.dma_start(maskc[:], maskd[:])
        idn = persist.tile([128, 128], BF16, tag="idn", name="idn")
        nc.scalar.dma_start(idn[:], idnd[:])
        bqk_sb = persist.tile([128, CCH], F32, tag="bqk", name="bqk_sb")
        nc.scalar.dma_start(bqk_sb[:], bqk.rearrange("(f p) -> p f", p=128))
        bvb_sb = persist.tile([128, HPC * EP], F32, tag="bvb", name="bvb_sb")
        nc.scalar.dma_start(bvb_sb[:], bvb[:])

        wqk_sb = [persist.tile([128, 2, FQK], FP8, tag=f"wqk{i}", name=f"wqk_sb{i}")
                  for i in range(CP)]
        # j-stride padded to 512 (dual-fp8 ISA: k-tile stride must be
        # 16-byte aligned)
        wv_sb = [persist.tile([128, 2, 512], FP8, tag=f"wv{i}", name=f"wv_sb{i}")
                 for i in range(CP)]
        wp_sb = [persist.tile([128, C], BF16, tag=f"wp{i}", name=f"wp_sb{i}")
                 for i in range(FV // 128)]
        for i in range(CP):
            for j in range(2):
                nc.gpsimd.dma_start(
                    wqk_sb[i][:, j, :],
                    wqk[i * 128:(i + 1) * 128, j * FQK:(j + 1) * FQK])
        for i in range(CP):
            for j in range(2):
                nc.gpsimd.dma_start(
                    wv_sb[i][:, j, 0:HPC * EP],
                    wv[i * 128:(i + 1) * 128, j * HPC * EP:(j + 1) * HPC * EP])
        for i in range(FV // 128):
            nc.gpsimd.dma_start(wp_sb[i][:], wp[i * 128:(i + 1) * 128, :])

        # x^T fp8, stored as chunk-pair tiles for DoubleRow
        xT = [persist.tile([128, 2, T], FP8, tag=f"xT{i}", name=f"xT{i}")
              for i in range(CP)]
        # QK^T bf16: tiles 0..2 hold Q^T (6 heads x 64), 3..5 hold K^T
        qkt = [persist.tile([128, T], BF16, tag=f"qkt{i}", name=f"qkt{i}")
               for i in range(CCH)]
        # V' fp8 pair tiles: [tok 128, kpair j, 6*65 feats] per k-block pair
        vp = [persist.tile([128, 2, 512], FP8, tag=f"vp{i}", name=f"vp{i}")
              for i in range(T // 256)]
        # bf16 V' for the first 4 token blocks (used by the q4=0 bf16 path)
        vpb = [persist.tile([128, HPC * EP], BF16, tag=f"vpb{i}", name=f"vpb{i}")
               for i in range(4)]
        # bf16 x^T (first 512 tokens) + bf16 wv for the q4=0 V projection
        xTb = [persist.tile([128, 512], BF16, tag=f"xTb{i}", name=f"xTb{i}")
               for i in range(CCH)]
        wvb_sb = [persist.tile([128, HPC * EP], BF16, tag=f"wvb{i}",
                               name=f"wvb_sb{i}") for i in range(CCH)]
        for i in range(CCH):
            nc.gpsimd.dma_start(wvb_sb[i][:], wvb[i * 128:(i + 1) * 128, :])
        for i in range(CCH):
            nc.gpsimd.dma_start(xTb[i][:], xbd[i * 128:(i + 1) * 128, :])

        def a_chunk(t4):
            # DMA this chunk's x^T columns (pre-transposed fp8 on host)
            for cp in range(CP):
                for j in range(2):
                    nc.sync.dma_start(
                        xT[cp][:, j, t4 * 512:(t4 + 1) * 512],
                        xtd[(2 * cp + j) * 128:(2 * cp + j + 1) * 128,
                            t4 * 512:(t4 + 1) * 512],
                    )

        def b_round(q4, r):
            # Q^T (ft=r) and K^T (ft=r+3) for head pair 2r, 2r+1
            for ft in (r, r + 3):
                ps = pvpp.tile([128, 512], F32, tag="pvpp", name="ps",
                               padded_shape=[128, 512])
                for cp in range(CP):
                    nc.tensor.matmul(
                        ps[:],
                        wqk_sb[cp][:, :, ft * 128:(ft + 1) * 128],
                        xT[cp][:, :, q4 * 512:(q4 + 1) * 512],
                        start=(cp == 0),
                        stop=(cp == CP - 1),
                        perf_mode=DR,
                    )
                nc.vector.tensor_scalar_add(
                    qkt[ft][:, q4 * 512:(q4 + 1) * 512],
                    ps[:],
                    bqk_sb[:, ft:ft + 1],
                )

        def c_chunk(t4):
            # V' fp8 pair tiles for 4 token blocks
            for j in range(4):
                tt = t4 * 4 + j
                pv = pvpp.tile([128, HPC * EP], F32, tag="pvpp", name="pv")
                for cp in range(CP):
                    nc.tensor.matmul(
                        pv[:],
                        xT[cp][:, :, tt * 128:(tt + 1) * 128],
                        wv_sb[cp][:, :, 0:HPC * EP],
                        start=(cp == 0),
                        stop=(cp == CP - 1),
                        perf_mode=DR,
                    )
                nc.vector.tensor_add(
                    vp[tt // 2][:, tt % 2, 0:HPC * EP], pv[:], bvb_sb[:])

        def ccb_unit(tt):
            # bf16 V' for one of token blocks 0..3 (q4=0 bf16 AV path)
            pv = pvpp.tile([128, HPC * EP], F32, tag="pvpp", name="pvb")
            for cc in range(CCH):
                nc.tensor.matmul(
                    pv[:],
                    xTb[cc][:, tt * 128:(tt + 1) * 128],
                    wvb_sb[cc][:],
                    start=(cc == 0),
                    stop=(cc == CCH - 1),
                )
            nc.vector.tensor_add(vpb[tt][:], pv[:], bvb_sb[:])

        def kt_slice(h, kb):
            return qkt[3 + h // 2][(h % 2) * 64:(h % 2) * 64 + 64,
                                   kb * 128:(kb + 1) * 128]

        def q_slice(h, q4, c0, c1):
            return qkt[h // 2][(h % 2) * 64:(h % 2) * 64 + 64,
                               q4 * 512 + c0:q4 * 512 + c1]

        def dummy(n=256):
            pv = pvpp.tile([128, 512], F32, tag="pvpp", name="dum")
            nc.tensor.matmul(pv[:, 0:n], idn[:], xTb[0][:, 0:n],
                             start=True, stop=True)

        def attn_heads(q4, yts, heads, fill=None):
            nkp = 2 * q4 + 2
            yzs = {h: psyz.tile([EP, 512], F32, tag="yz", name=f"yz{h}")
                   for h in heads}
            prev_av = None
            for kp in range(nkp):
                po = max(0, kp * 256 - q4 * 512)  # pair col offset
                for h in heads:
                    sp = spp.tile([128, 2, 512], F32, tag="spp", name="sp")
                    for jj in range(2):
                        kb = 2 * kp + jj
                        off = max(0, kb * 128 - q4 * 512)
                        diag = kb * 128 >= q4 * 512
                        if diag:
                            # preset masked region with NEG (rect for the
                            # pair-mate gap of odd blocks, triangle at off),
                            # then accumulate S on top of it; cols beyond the
                            # preset get their own start=True matmul
                            p0 = po if jj == 1 else off
                            nc.tensor.matmul(
                                sp[:, jj, p0:off + 128],
                                idn[:],
                                maskc[:, 128 - (off - p0):256],
                                start=True, stop=False,
                            )
                            nc.tensor.matmul(
                                sp[:, jj, off:off + 128],
                                kt_slice(h, kb),
                                q_slice(h, q4, off, off + 128),
                                start=False, stop=True,
                            )
                            if off + 128 < 512:
                                nc.tensor.matmul(
                                    sp[:, jj, off + 128:512],
                                    kt_slice(h, kb),
                                    q_slice(h, q4, off + 128, 512),
                                    start=True, stop=True,
                                )
                        else:
                            nc.tensor.matmul(
                                sp[:, jj, 0:512],
                                kt_slice(h, kb),
                                q_slice(h, q4, 0, 512),
                                start=True, stop=True,
                            )
                    if q4 == 0:
                        # bf16 z + per-block bf16 AV (small-q rows average
                        # over too few tokens to tolerate fp8 noise)
                        zt = zpoolb.tile([128, 2, 512], BF16, tag="ztb",
                                         name="ztb")
                        nc.scalar.activation(
                            zt[:, :, po:512], sp[:, :, po:512],
                            mybir.ActivationFunctionType.Exp,
                            scale=float(EXPS),
                        )

                        def av(zt=zt, kp=kp, h=h):
                            for jj in range(2):
                                kb = 2 * kp + jj
                                off = kb * 128
                                nc.tensor.matmul(
                                    yzs[h][:, off:512],
                                    vpb[kb][:, h * EP:(h + 1) * EP],
                                    zt[:, jj, off:512],
                                    start=(kb == 0), stop=(kb == 3),
                                )
                    else:
                        zt = zpool.tile([128, 2, 512], FP8, tag="zt", name="zt")
                        nc.scalar.activation(
                            zt[:, :, po:512], sp[:, :, po:512],
                            mybir.ActivationFunctionType.Exp,
                            scale=float(EXPS),
                        )

                        def av(zt=zt, kp=kp, h=h, po=po):
                            nc.tensor.matmul(
                                yzs[h][:, po:512],
                                vp[kp][:, :, h * EP:(h + 1) * EP],
                                zt[:, :, po:512],
                                start=(kp == 0), stop=(kp == nkp - 1),
                                perf_mode=DR,
                            )
                    # deferred AV: emit the PREVIOUS group's AV so its exp has
                    # had a full group's worth of PE time to complete
                    if prev_av is not None:
                        prev_av()
                    prev_av = av
                    if fill is None or not fill.tick():
                        # keep the HAM activity monitor warm: PE must stay
                        # saturated or it drops to 1.2GHz and sticks there
                        dummy(512)
                    if q4 == 3:
                        dummy(512)
            if prev_av is not None:
                prev_av()
            dens, rcs, bcs = {}, {}, {}
            for h in heads:
                dens[h] = spool.tile([1, 512], F32, tag="den0", name="den0")
                nc.vector.tensor_copy(dens[h][:], yzs[h][64:65, :])
            dummy()
            for h in heads:
                rcs[h] = spool.tile([1, 512], F32, tag="rc", name="rc")
                nc.vector.reciprocal_approx_fast(rcs[h][:], dens[h][:])
            for h in heads:
                bcs[h] = spool.tile([64, 512], F32, tag="bc_sb", name="bc_sb")
                nc.gpsimd.partition_broadcast(bcs[h][:], rcs[h][:])
            dummy()
            for h in heads:
                nc.vector.tensor_mul(
                    yts[h // 2][(h % 2) * 64:(h % 2) * 64 + 64, :],
                    yzs[h][0:64, :], bcs[h][:],
                )

        def proj_unit(q4, yts, qt):
            ot = opool.tile([128, C], BF16, tag="ot", name="ot")
            for half in range(2):
                pp = pvpp.tile([128, 512], F32, tag="pvpp", name="pp")
                for hdc in range(FV // 128):
                    nc.tensor.matmul(
                        pp[:, 0:384],
                        yts[hdc][:, qt * 128:(qt + 1) * 128],
                        wp_sb[hdc][:, half * 384:(half + 1) * 384],
                        start=(hdc == 0), stop=(hdc == FV // 128 - 1),
                    )
                nc.vector.tensor_copy(
                    ot[:, half * 384:(half + 1) * 384], pp[:, 0:384])
            row = (q4 * 4 + qt) * 128
            nc.sync.dma_start(yp[row:row + 128, :], ot[:])

        class Fill:
            """Evenly spreads independent PE work units across the attention
            groups so the tensor engine never idles waiting on exp."""

            def __init__(self, units, total_groups):
                self.units = list(units)
                self.n0 = len(self.units)
                self.total = max(1, total_groups)
                self.seen = 0

            def tick(self):
                self.seen += 1
                want = self.n0 * self.seen // self.total
                done = self.n0 - len(self.units)
                emitted = False
                while done < want and self.units:
                    self.units.pop(0)()
                    done += 1
                    emitted = True
                return emitted

            def drain(self):
                while self.units:
                    self.units.pop(0)()

        def cc_unit(t4, j):
            tt = t4 * 4 + j
            pv = pvpp.tile([128, HPC * EP], F32, tag="pvpp", name="pv")
            for cp in range(CP):
                nc.tensor.matmul(
                    pv[:],
                    xT[cp][:, :, tt * 128:(tt + 1) * 128],
                    wv_sb[cp][:, :, 0:HPC * EP],
                    start=(cp == 0),
                    stop=(cp == CP - 1),
                    perf_mode=DR,
                )
            nc.vector.tensor_add(
                vp[tt // 2][:, tt % 2, 0:HPC * EP], pv[:], bvb_sb[:])

        # prologue: x chunk 0, Q/K for chunk 0, bf16 V' (needed by q4=0 attn)
        a_chunk(0)
        for r in range(3):
            b_round(0, r)
        ccb_unit(0)
        ccb_unit(1)
        pending = None
        for q4 in range(QC):
            if q4 + 1 < QC:
                a_chunk(q4 + 1)
            yts = [ypool.tile([128, 512], BF16, tag=f"yt{i}", name=f"yt{i}")
                   for i in range(3)]
            units = []
            if pending is not None:
                pq4, pyts = pending
                units += [
                    (lambda qt=qt, pq4=pq4, pyts=pyts: proj_unit(pq4, pyts, qt))
                    for qt in range(4)]
            if q4 == 0:
                units += [(lambda tt=tt: ccb_unit(tt)) for tt in (2, 3)]
                units += [(lambda j=j: cc_unit(0, j)) for j in range(4)]
            if q4 + 1 < QC:
                br = [(lambda r=r: b_round(q4 + 1, r)) for r in range(3)]
                cc = [(lambda j=j: cc_unit(q4 + 1, j)) for j in range(4)]
                # round-robin the unit kinds
                mix = []
                while br or cc:
                    if cc:
                        mix.append(cc.pop(0))
                    if br:
                        mix.append(br.pop(0))
                units += mix
            fill = Fill(units, (2 * q4 + 2) * 6)
            attn_heads(q4, yts, [0, 1], fill)
            attn_heads(q4, yts, [2, 3], fill)
            attn_heads(q4, yts, [4, 5], fill)
            fill.drain()
            if dbg is not None and q4 == 0:
                nc.sync.dma_start(dbg["dqkt0"][:, 0:512], qkt[0][:, 0:512])
                nc.sync.dma_start(dbg["dqkt3"][:, 0:512], qkt[3][:, 0:512])
                nc.sync.dma_start(
                    dbg["dvp0"][:, 0:HPC * EP], vp[0][:, 0, 0:HPC * EP])
                nc.sync.dma_start(
                    dbg["dvp0"][:, 512:512 + HPC * EP], vp[0][:, 1, 0:HPC * EP])
                nc.sync.dma_start(dbg["dyts0"][:], yts[0][:])
                nc.sync.dma_start(dbg["dyts1"][:], yts[1][:])
            pending = (q4, yts)
        for qt in range(4):
            proj_unit(pending[0], pending[1], qt)


_PROGRAM = None


def _get_program():
    global _PROGRAM
    if _PROGRAM is None:
        _PROGRAM = _build_program()
    return _PROGRAM


def _pack_pairs(w):
    """[C, F] -> [CP*128, 2*F]: row r=cp*128+p, col j*F+f = w[(2cp+j)*128+p, f]."""
    F = w.shape[1]
    out = np.zeros((CP * 128, 2 * F), dtype=w.dtype)
    for cp in range(CP):
        for j in range(2):
            out[cp * 128:(cp + 1) * 128, j * F:(j + 1) * F] = \
                w[(2 * cp + j) * 128:(2 * cp + j + 1) * 128, :]
    return out


def _pad_wv(wv):
    out = np.zeros((C, HPC * EP), dtype=np.float32)
    for h in range(HPC):
        out[:, h * EP:h * EP + D] = wv[:, h * D:(h + 1) * D]
    return out


def kernel(x, W_attn, b_attn, W_proj, b_proj):
    x = np.ascontiguousarray(x, dtype=np.float32)
    W_attn = np.ascontiguousarray(W_attn, dtype=np.float32)
    b_attn = np.ascontiguousarray(b_attn, dtype=np.float32)
    W_proj = np.ascontiguousarray(W_proj, dtype=np.float32)
    b_proj = np.ascontiguousarray(b_proj, dtype=np.float32)

    nc = _get_program()
    mask_const = np.ones((128, 256), np.float32)
    mask_const[:, 128:] = np.tril(np.ones((128, 128), np.float32), -1)
    mask_const = mask_const.astype(NPBF)
    idn_const = (NEG * np.eye(128, dtype=np.float32)).astype(NPBF)

    in_maps = []
    for core in range(N_CORES):
        b, g = core // 2, core % 2
        qcols = slice(384 * g, 384 * (g + 1))
        kcols = slice(768 + 384 * g, 768 + 384 * (g + 1))
        vcols = slice(1536 + 384 * g, 1536 + 384 * (g + 1))
        wqk_full = np.concatenate(
            [W_attn[:, qcols], W_attn[:, kcols]], axis=1) * WS
        wv_full = _pad_wv(W_attn[:, vcols]) * WS
        bvb_row = np.zeros((HPC * EP,), np.float32)
        for h in range(HPC):
            bvb_row[h * EP:h * EP + D] = WS * b_attn[vcols][h * D:(h + 1) * D]
            bvb_row[h * EP + D] = WS
        xt = np.ascontiguousarray(x[b].T)
        in_maps.append({
            "xtd": xt.astype(NPF8),
            "xbd": np.ascontiguousarray(xt[:, :512]).astype(NPBF),
            "wqk": _pack_pairs(wqk_full.astype(NPF8)),
            "wv": _pack_pairs(wv_full.astype(NPF8)),
            "wvb": wv_full.astype(NPBF),
            "bqk": np.ascontiguousarray(
                WS * np.concatenate([b_attn[qcols], b_attn[kcols]])),
            "bvb": np.ascontiguousarray(
                np.broadcast_to(bvb_row, (128, HPC * EP))),
            "wp": np.ascontiguousarray(
                W_proj[384 * g:384 * (g + 1), :]).astype(NPBF),
            "maskd": mask_const,
            "idnd": idn_const,
        })

    trace = bool(int(os.environ.get("KBENCH_TRACE", "0")))
    if trace:
        _install_ntff_hook()
    res = run_bass_kernel_spmd(
        nc, in_maps, list(range(N_CORES)), trace=trace,
    )
    kernel.last_exec_time_ns = res.exec_time_ns

    out = np.empty((B, T, C), dtype=np.float32)
    for b in range(B):
        out[b] = (res.results[2 * b]["yp"].astype(np.float32)
                  + res.results[2 * b + 1]["yp"].astype(np.float32) + b_proj)
    return out


# revision 33
# speedup vs baseline: 1.0290x; 1.0040x over previous
"""Causal self-attention (B=4, T=2048, C=768, H=12) on 8 trn2 NeuronCores.

Sharding: 8 cores = 4 batches x 2 head-groups (6 heads each).
Each core: QKV projection for its 6 heads, causal attention, partial output
projection (row-parallel). Host sums the two partials per batch + b_proj.

Device-side dataflow (fp8 DoubleRow + HAM-aware software pipelining):
  - x shipped as x^T fp8e4 (values ~N(0,1), well inside e4m3 range).
  - QKV weights host-scaled by 32 (avoids fp8 subnormals) and packed in
    contraction PAIRS for DoubleRow matmuls: lhsT/rhs APs [128, 2, M].
    Q,K come out x32; the 1/32^2 is folded into the exp scale (1/8192).
    V comes out x32 with a x32 ones column, so y = (32*sum z v)/(32*sum z)
    is exactly unscaled.
  - Causal mask: PE "preset" matmuls accumulate -245760 * [ones|L] into the
    masked PSUM region before the S matmul lands; exp(x/8192) then flushes
    those entries to exactly 0 in fp8. No vector-engine masking at all.
  - S^T computed per k-block into [128, 2, 512] PSUM pair tiles (2 banks);
    ONE exp activation per (k-pair, head) writes the fp8 z pair tile.
  - AV matmul in fp8 DoubleRow over k-block pairs; the V' ones column
    yields the softmax denominator for free. y normalized via
    reciprocal_approx_fast + gpsimd partition_broadcast + vector multiply.
  - q4=0 (first 512 queries) runs attention in bf16: those rows average
    over too few tokens for fp8 noise to cancel (fp8 there costs ~3e-2
    max-rel error; bf16 keeps the whole kernel at ~9e-3).
  - Output projection in bf16, result DMA'd out as bf16 partials.
  - Scheduling: the PE HAM clock gate halves the PE clock unless the engine
    stays saturated, so attention groups (S-pair -> exp -> AV) are software
    pipelined (AV deferred one group), QKV/V'/output-projection work is
    spread across the attention groups as filler units, and small dummy
    matmuls pad any remaining PE idle to keep the clock at 2.4GHz.
"""

import os
import sys
import types

sys.path.insert(0, "/opt/trn_rl_repo")

import ml_dtypes
import numpy as np

import concourse.bass as bass
import concourse.tile as tile
from concourse import bacc, mybir
from concourse.bass_utils import run_bass_kernel_spmd

B, T, C, H, D = 4, 2048, 768, 12, 64
N_CORES = 8
HPC = H // 2          # heads per core = 6
FQK = 2 * HPC * D     # 768 qk features per core
FV = HPC * D          # 384 v features per core
E = D + 1             # 65: head dim + ones column
EP = 68               # padded V' per-head width (dual-fp8 ldweights alignment)
CCH = C // 128        # 6 contraction chunks
CP = CCH // 2         # 3 contraction chunk pairs (DoubleRow)
QC = T // 512         # 4 query chunks of 512
F32 = mybir.dt.float32
BF16 = mybir.dt.bfloat16
FP8 = mybir.dt.float8e4
NPBF = ml_dtypes.bfloat16
NPF8 = ml_dtypes.float8_e4m3
DR = mybir.MatmulPerfMode.DoubleRow

WS = 32.0             # host weight scale (fp8 subnormal avoidance)
EXPS = 1.0 / (np.sqrt(D) * WS * WS)   # 1/(8*1024) = 1/8192
NEG = -30.0 * WS * WS * np.sqrt(D)    # -245760; exp(NEG*EXPS) == exp(-30)


def _install_ntff_hook():
    """The image's antenv lacks axon_hooks; inject it so trace=True works."""
    if "antenv.axon_hooks" in sys.modules:
        return
    try:
        import antenv
        mod = types.ModuleType("antenv.axon_hooks")
        _state = {"hook": None}
        mod.set_axon_ntff_profile_hook = lambda h: _state.__setitem__("hook", h)
        mod.get_axon_ntff_profile_hook = lambda: _state["hook"]
        sys.modules["antenv.axon_hooks"] = mod
        antenv.axon_hooks = mod
        from trn_agent_boot.trn_boot import _ntff_profile_via_ctypes
        mod.set_axon_ntff_profile_hook(
            _ntff_profile_via_ctypes("/opt/axon/libaxon_pjrt.so")
        )
    except Exception:
        pass


def _build_program():
    nc = bacc.Bacc(
        "TRN2",
        target_bir_lowering=False,
        debug=False,
        enable_asserts=False,
        num_devices=N_CORES,
    )
    xtd = nc.dram_tensor("xtd", [C, T], FP8, kind="ExternalInput").ap()
    xbd = nc.dram_tensor("xbd", [C, 512], BF16, kind="ExternalInput").ap()
    wqk = nc.dram_tensor("wqk", [CP * 128, 2 * FQK], FP8, kind="ExternalInput").ap()
    wv = nc.dram_tensor("wv", [CP * 128, 2 * HPC * EP], FP8, kind="ExternalInput").ap()
    wvb = nc.dram_tensor("wvb", [C, HPC * EP], BF16, kind="ExternalInput").ap()
    bqk = nc.dram_tensor("bqk", [FQK], F32, kind="ExternalInput").ap()
    bvb = nc.dram_tensor("bvb", [128, HPC * EP], F32, kind="ExternalInput").ap()
    wp = nc.dram_tensor("wp", [FV, C], BF16, kind="ExternalInput").ap()
    maskd = nc.dram_tensor("maskd", [128, 256], BF16, kind="ExternalInput").ap()
    idnd = nc.dram_tensor("idnd", [128, 128], BF16, kind="ExternalInput").ap()
    yp = nc.dram_tensor("yp", [T, C], BF16, kind="ExternalOutput").ap()
    dbg = None
    if os.environ.get("KDEBUG"):
        dbg = {
            "dqkt0": nc.dram_tensor("dqkt0", [128, T], BF16,
                                    kind="ExternalOutput").ap(),
            "dqkt3": nc.dram_tensor("dqkt3", [128, T], BF16,
                                    kind="ExternalOutput").ap(),
            "dvp0": nc.dram_tensor("dvp0", [128, 1024], FP8,
                                   kind="ExternalOutput").ap(),
            "dyts0": nc.dram_tensor("dyts0", [128, 512], BF16,
                                    kind="ExternalOutput").ap(),
            "dyts1": nc.dram_tensor("dyts1", [128, 512], BF16,
                                    kind="ExternalOutput").ap(),
            "dsp": nc.dram_tensor("dsp", [128, 1024], F32,
                                  kind="ExternalOutput").ap(),
            "dzt": nc.dram_tensor("dzt", [128, 1024], BF16,
                                  kind="ExternalOutput").ap(),
            "dyz": nc.dram_tensor("dyz", [EP, 512], F32,
                                  kind="ExternalOutput").ap(),
            "drc": nc.dram_tensor("drc", [1, 512], F32,
                                  kind="ExternalOutput").ap(),
        }

    with tile.TileContext(nc) as tc:
        _body(tc, nc, xtd, xbd, wqk, wv, wvb, bqk, bvb, wp, maskd, idnd, yp,
              dbg)

    nc.compile()
    return nc


def _body(tc, nc, xtd, xbd, wqk, wv, wvb, bqk, bvb, wp, maskd, idnd, yp,
          dbg=None):
    from contextlib import ExitStack

    with ExitStack() as es:
        persist = es.enter_context(tc.tile_pool(name="persist", bufs=1))
        # S pair tiles + QKV-proj pair tiles share one 2-bank-tile pool
        spp = es.enter_context(tc.tile_pool(name="spp", bufs=2, space="PSUM"))
        pvpp = es.enter_context(tc.tile_pool(name="pvpp", bufs=2, space="PSUM"))
        psyz = es.enter_context(tc.tile_pool(name="psyz", bufs=2, space="PSUM"))
        zpool = es.enter_context(tc.tile_pool(name="zpool", bufs=5))
        zpoolb = es.enter_context(tc.tile_pool(name="zpoolb", bufs=3))
        ypool = es.enter_context(tc.tile_pool(name="ypool", bufs=2))
        opool = es.enter_context(tc.tile_pool(name="opool", bufs=3))
        spool = es.enter_context(tc.tile_pool(name="spool", bufs=4))

        # ---- small constants first (scalar queue) so big loads start early
        maskc = persist.tile([128, 256], BF16, tag="maskc", name="maskc")
        nc.scalar.dma_start(maskc[:], maskd[:])
        idn = persist.tile([128, 128], BF16, tag="idn", name="idn")
        nc.scalar.dma_start(idn[:], idnd[:])
        bqk_sb = persist.tile([128, CCH], F32, tag="bqk", name="bqk_sb")
        nc.scalar.dma_start(bqk_sb[:], bqk.rearrange("(f p) -> p f", p=128))
        bvb_sb = persist.tile([128, HPC * EP], F32, tag="bvb", name="bvb_sb")
        nc.scalar.dma_start(bvb_sb[:], bvb[:])

        wqk_sb = [persist.tile([128, 2, FQK], FP8, tag=f"wqk{i}", name=f"wqk_sb{i}")
                  for i in range(CP)]
        # j-stride padded to 512 (dual-fp8 ISA: k-tile stride must be
        # 16-byte aligned)
        wv_sb = [persist.tile([128, 2, 512], FP8, tag=f"wv{i}", name=f"wv_sb{i}")
                 for i in range(CP)]
        wp_sb = [persist.tile([128, C], BF16, tag=f"wp{i}", name=f"wp_sb{i}")
                 for i in range(FV // 128)]
        for i in range(CP):
            for j in range(2):
                nc.gpsimd.dma_start(
                    wqk_sb[i][:, j, :],
                    wqk[i * 128:(i + 1) * 128, j * FQK:(j + 1) * FQK])
        for i in range(CP):
            for j in range(2):
                nc.gpsimd.dma_start(
                    wv_sb[i][:, j, 0:HPC * EP],
                    wv[i * 128:(i + 1) * 128, j * HPC * EP:(j + 1) * HPC * EP])
        for i in range(FV // 128):
            nc.gpsimd.dma_start(wp_sb[i][:], wp[i * 128:(i + 1) * 128, :])

        # x^T fp8, stored as chunk-pair tiles for DoubleRow
        xT = [persist.tile([128, 2, T], FP8, tag=f"xT{i}", name=f"xT{i}")
              for i in range(CP)]
        # QK^T bf16: tiles 0..2 hold Q^T (6 heads x 64), 3..5 hold K^T
        qkt = [persist.tile([128, T], BF16, tag=f"qkt{i}", name=f"qkt{i}")
               for i in range(CCH)]
        # V' fp8 pair tiles: [tok 128, kpair j, 6*65 feats] per k-block pair
        vp = [persist.tile([128, 2, 512], FP8, tag=f"vp{i}", name=f"vp{i}")
              for i in range(T // 256)]
        # bf16 V' for the first 4 token blocks (used by the q4=0 bf16 path)
        vpb = [persist.tile([128, HPC * EP], BF16, tag=f"vpb{i}", name=f"vpb{i}")
               for i in range(4)]
        # bf16 x^T (first 512 tokens) + bf16 wv for the q4=0 V projection
        xTb = [persist.tile([128, 512], BF16, tag=f"xTb{i}", name=f"xTb{i}")
               for i in range(CCH)]
        wvb_sb = [persist.tile([128, HPC * EP], BF16, tag=f"wvb{i}",
                               name=f"wvb_sb{i}") for i in range(CCH)]
        for i in range(CCH):
            nc.gpsimd.dma_start(wvb_sb[i][:], wvb[i * 128:(i + 1) * 128, :])
        for i in range(CCH):
            nc.gpsimd.dma_start(xTb[i][:], xbd[i * 128:(i + 1) * 128, :])

        def a_chunk(t4):
            # DMA this chunk's x^T columns (pre-transposed fp8 on host)
            for cp in range(CP):
                for j in range(2):
                    nc.sync.dma_start(
                        xT[cp][:, j, t4 * 512:(t4 + 1) * 512],
                        xtd[(2 * cp + j) * 128:(2 * cp + j + 1) * 128,
                            t4 * 512:(t4 + 1) * 512],
                    )

        def b_round(q4, r):
            # Q^T (ft=r) and K^T (ft=r+3) for head pair 2r, 2r+1
            for ft in (r, r + 3):
                ps = pvpp.tile([128, 512], F32, tag="pvpp", name="ps",
                               padded_shape=[128, 512])
                for cp in range(CP):
                    nc.tensor.matmul(
                        ps[:],
                        wqk_sb[cp][:, :, ft * 128:(ft + 1) * 128],
                        xT[cp][:, :, q4 * 512:(q4 + 1) * 512],
                        start=(cp == 0),
                        stop=(cp == CP - 1),
                        perf_mode=DR,
                    )
                nc.vector.tensor_scalar_add(
                    qkt[ft][:, q4 * 512:(q4 + 1) * 512],
                    ps[:],
                    bqk_sb[:, ft:ft + 1],
                )

        def ccb_unit(tt):
            # bf16 V' for one of token blocks 0..3 (q4=0 bf16 AV path)
            pv = pvpp.tile([128, HPC * EP], F32, tag="pvpp", name="pvb")
            for cc in range(CCH):
                nc.tensor.matmul(
                    pv[:],
                    xTb[cc][:, tt * 128:(tt + 1) * 128],
                    wvb_sb[cc][:],
                    start=(cc == 0),
                    stop=(cc == CCH - 1),
                )
            nc.vector.tensor_add(vpb[tt][:], pv[:], bvb_sb[:])

        def kt_slice(h, kb):
            return qkt[3 + h // 2][(h % 2) * 64:(h % 2) * 64 + 64,
                                   kb * 128:(kb + 1) * 128]

        def q_slice(h, q4, c0, c1):
            return qkt[h // 2][(h % 2) * 64:(h % 2) * 64 + 64,
                               q4 * 512 + c0:q4 * 512 + c1]

        def dummy(n=256):
            pv = pvpp.tile([128, 512], F32, tag="pvpp", name="dum")
            nc.tensor.matmul(pv[:, 0:n], idn[:], xTb[0][:, 0:n],
                             start=True, stop=True)

        def attn_heads(q4, yts, heads, fill=None):
            nkp = 2 * q4 + 2
            yzs = {h: psyz.tile([EP, 512], F32, tag="yz", name=f"yz{h}")
                   for h in heads}
            prev_av = None
            for kp in range(nkp):
                po = max(0, kp * 256 - q4 * 512)  # pair col offset
                for h in heads:
                    sp = spp.tile([128, 2, 512], F32, tag="spp", name="sp")
                    for jj in range(2):
                        kb = 2 * kp + jj
                        off = max(0, kb * 128 - q4 * 512)
                        diag = kb * 128 >= q4 * 512
                        if diag:
                            # preset masked region with NEG (rect for the
                            # pair-mate gap of odd blocks, triangle at off),
                            # then accumulate S on top of it; cols beyond the
                            # preset get their own start=True matmul
                            p0 = po if jj == 1 else off
                            nc.tensor.matmul(
                                sp[:, jj, p0:off + 128],
                                idn[:],
                                maskc[:, 128 - (off - p0):256],
                                start=True, stop=False,
                            )
                            nc.tensor.matmul(
                                sp[:, jj, off:off + 128],
                                kt_slice(h, kb),
                                q_slice(h, q4, off, off + 128),
                                start=False, stop=True,
                            )
                            if off + 128 < 512:
                                nc.tensor.matmul(
                                    sp[:, jj, off + 128:512],
                                    kt_slice(h, kb),
                                    q_slice(h, q4, off + 128, 512),
                                    start=True, stop=True,
                                )
                        else:
                            nc.tensor.matmul(
                                sp[:, jj, 0:512],
                                kt_slice(h, kb),
                                q_slice(h, q4, 0, 512),
                                start=True, stop=True,
                            )
                    if q4 == 0:
                        # bf16 z + per-block bf16 AV (small-q rows average
                        # over too few tokens to tolerate fp8 noise)
                        zt = zpoolb.tile([128, 2, 512], BF16, tag="ztb",
                                         name="ztb")
                        nc.scalar.activation(
                            zt[:, :, po:512], sp[:, :, po:512],
                            mybir.ActivationFunctionType.Exp,
                            scale=float(EXPS),
                        )

                        def av(zt=zt, kp=kp, h=h):
                            for jj in range(2):
                                kb = 2 * kp + jj
                                off = kb * 128
                                nc.tensor.matmul(
                                    yzs[h][:, off:512],
                                    vpb[kb][:, h * EP:(h + 1) * EP],
                                    zt[:, jj, off:512],
                                    start=(kb == 0), stop=(kb == 3),
                                )
                    else:
                        zt = zpool.tile([128, 2, 512], FP8, tag="zt", name="zt")
                        nc.scalar.activation(
                            zt[:, :, po:512], sp[:, :, po:512],
                            mybir.ActivationFunctionType.Exp,
                            scale=float(EXPS),
                        )

                        def av(zt=zt, kp=kp, h=h, po=po):
                            nc.tensor.matmul(
                                yzs[h][:, po:512],
                                vp[kp][:, :, h * EP:(h + 1) * EP],
                                zt[:, :, po:512],
                                start=(kp == 0), stop=(kp == nkp - 1),
                                perf_mode=DR,
                            )
                    # deferred AV: emit the PREVIOUS group's AV so its exp has
                    # had a full group's worth of PE time to complete
                    if prev_av is not None:
                        prev_av()
                    prev_av = av
                    if fill is None or not fill.tick():
                        # keep the HAM activity monitor warm: PE must stay
                        # saturated or it drops to 1.2GHz and sticks there
                        dummy(512)
                    if q4 == 3:
                        dummy(512)
            if prev_av is not None:
                prev_av()
            dens, rcs, bcs = {}, {}, {}
            for h in heads:
                dens[h] = spool.tile([1, 512], F32, tag="den0", name="den0")
                nc.vector.tensor_copy(dens[h][:], yzs[h][64:65, :])
            dummy()
            for h in heads:
                rcs[h] = spool.tile([1, 512], F32, tag="rc", name="rc")
                nc.vector.reciprocal_approx_fast(rcs[h][:], dens[h][:])
            for h in heads:
                bcs[h] = spool.tile([64, 512], F32, tag="bc_sb", name="bc_sb")
                nc.gpsimd.partition_broadcast(bcs[h][:], rcs[h][:])
            dummy()
            for h in heads:
                nc.vector.tensor_mul(
                    yts[h // 2][(h % 2) * 64:(h % 2) * 64 + 64, :],
                    yzs[h][0:64, :], bcs[h][:],
                )

        def proj_unit(q4, yts, qt):
            ot = opool.tile([128, C], BF16, tag="ot", name="ot")
            for half in range(2):
                pp = pvpp.tile([128, 512], F32, tag="pvpp", name="pp")
                for hdc in range(FV // 128):
                    nc.tensor.matmul(
                        pp[:, 0:384],
                        yts[hdc][:, qt * 128:(qt + 1) * 128],
                        wp_sb[hdc][:, half * 384:(half + 1) * 384],
                        start=(hdc == 0), stop=(hdc == FV // 128 - 1),
                    )
                nc.vector.tensor_copy(
                    ot[:, half * 384:(half + 1) * 384], pp[:, 0:384])
            row = (q4 * 4 + qt) * 128
            nc.sync.dma_start(yp[row:row + 128, :], ot[:])

        class Fill:
            """Evenly spreads independent PE work units across the attention
            groups so the tensor engine never idles waiting on exp."""

            def __init__(self, units, total_groups):
                self.units = list(units)
                self.n0 = len(self.units)
                self.total = max(1, total_groups)
                self.seen = 0

            def tick(self):
                self.seen += 1
                want = self.n0 * self.seen // self.total
                done = self.n0 - len(self.units)
                emitted = False
                while done < want and self.units:
                    self.units.pop(0)()
                    done += 1
                    emitted = True
                return emitted

            def drain(self):
                while self.units:
                    self.units.pop(0)()

        def cc_unit(t4, j):
            tt = t4 * 4 + j
            pv = pvpp.tile([128, HPC * EP], F32, tag="pvpp", name="pv")
            for cp in range(CP):
                nc.tensor.matmul(
                    pv[:],
                    xT[cp][:, :, tt * 128:(tt + 1) * 128],
                    wv_sb[cp][:, :, 0:HPC * EP],
                    start=(cp == 0),
                    stop=(cp == CP - 1),
                    perf_mode=DR,
                )
            nc.vector.tensor_add(
                vp[tt // 2][:, tt % 2, 0:HPC * EP], pv[:], bvb_sb[:])

        # prologue: x chunk 0, Q/K for chunk 0, bf16 V' (needed by q4=0 attn)
        a_chunk(0)
        for r in range(3):
            b_round(0, r)
        ccb_unit(0)
        ccb_unit(1)
        pending = None
        for q4 in range(QC):
            if q4 + 1 < QC:
                a_chunk(q4 + 1)
            yts = [ypool.tile([128, 512], BF16, tag=f"yt{i}", name=f"yt{i}")
                   for i in range(3)]
            units = []
            if pending is not None:
                pq4, pyts = pending
                units += [
                    (lambda qt=qt, pq4=pq4, pyts=pyts: proj_unit(pq4, pyts, qt))
                    for qt in range(4)]
            if q4 == 0:
                units += [(lambda tt=tt: ccb_unit(tt)) for tt in (2, 3)]
                units += [(lambda j=j: cc_unit(0, j)) for j in range(4)]
            if q4 + 1 < QC:
                br = [(lambda r=r: b_round(q4 + 1, r)) for r in range(3)]
                cc = [(lambda j=j: cc_unit(q4 + 1, j)) for j in range(4)]
                # round-robin the unit kinds
                mix = []
                while br or cc:
                    if cc:
                        mix.append(cc.pop(0))
                    if br:
                        mix.append(br.pop(0))
                units += mix
            fill = Fill(units, (2 * q4 + 2) * 6)
            attn_heads(q4, yts, [0, 1], fill)
            attn_heads(q4, yts, [2, 3], fill)
            attn_heads(q4, yts, [4, 5], fill)
            fill.drain()
            if dbg is not None and q4 == 0:
                nc.sync.dma_start(dbg["dqkt0"][:, 0:512], qkt[0][:, 0:512])
                nc.sync.dma_start(dbg["dqkt3"][:, 0:512], qkt[3][:, 0:512])
                nc.sync.dma_start(
                    dbg["dvp0"][:, 0:HPC * EP], vp[0][:, 0, 0:HPC * EP])
                nc.sync.dma_start(
                    dbg["dvp0"][:, 512:512 + HPC * EP], vp[0][:, 1, 0:HPC * EP])
                nc.sync.dma_start(dbg["dyts0"][:], yts[0][:])
                nc.sync.dma_start(dbg["dyts1"][:], yts[1][:])
            pending = (q4, yts)
        for qt in range(4):
            proj_unit(pending[0], pending[1], qt)


_PROGRAM = None


def _get_program():
    global _PROGRAM
    if _PROGRAM is None:
        _PROGRAM = _build_program()
    return _PROGRAM


def _pack_pairs(w):
    """[C, F] -> [CP*128, 2*F]: row r=cp*128+p, col j*F+f = w[(2cp+j)*128+p, f]."""
    F = w.shape[1]
    out = np.zeros((CP * 128, 2 * F), dtype=w.dtype)
    for cp in range(CP):
        for j in range(2):
            out[cp * 128:(cp + 1) * 128, j * F:(j + 1) * F] = \
                w[(2 * cp + j) * 128:(2 * cp + j + 1) * 128, :]
    return out


def _pad_wv(wv):
    out = np.zeros((C, HPC * EP), dtype=np.float32)
    for h in range(HPC):
        out[:, h * EP:h * EP + D] = wv[:, h * D:(h + 1) * D]
    return out


def kernel(x, W_attn, b_attn, W_proj, b_proj):
    x = np.ascontiguousarray(x, dtype=np.float32)
    W_attn = np.ascontiguousarray(W_attn, dtype=np.float32)
    b_attn = np.ascontiguousarray(b_attn, dtype=np.float32)
    W_proj = np.ascontiguousarray(W_proj, dtype=np.float32)
    b_proj = np.ascontiguousarray(b_proj, dtype=np.float32)

    nc = _get_program()
    mask_const = np.ones((128, 256), np.float32)
    mask_const[:, 128:] = np.tril(np.ones((128, 128), np.float32), -1)
    mask_const = mask_const.astype(NPBF)
    idn_const = (NEG * np.eye(128, dtype=np.float32)).astype(NPBF)

    in_maps = []
    for core in range(N_CORES):
        b, g = core // 2, core % 2
        qcols = slice(384 * g, 384 * (g + 1))
        kcols = slice(768 + 384 * g, 768 + 384 * (g + 1))
        vcols = slice(1536 + 384 * g, 1536 + 384 * (g + 1))
        wqk_full = np.concatenate(
            [W_attn[:, qcols], W_attn[:, kcols]], axis=1) * WS
        wv_full = _pad_wv(W_attn[:, vcols]) * WS
        bvb_row = np.zeros((HPC * EP,), np.float32)
        for h in range(HPC):
            bvb_row[h * EP:h * EP + D] = WS * b_attn[vcols][h * D:(h + 1) * D]
            bvb_row[h * EP + D] = WS
        xt = np.ascontiguousarray(x[b].T)
        in_maps.append({
            "xtd": xt.astype(NPF8),
            "xbd": np.ascontiguousarray(xt[:, :512]).astype(NPBF),
            "wqk": _pack_pairs(wqk_full.astype(NPF8)),
            "wv": _pack_pairs(wv_full.astype(NPF8)),
            "wvb": wv_full.astype(NPBF),
            "bqk": np.ascontiguousarray(
                WS * np.concatenate([b_attn[qcols], b_attn[kcols]])),
            "bvb": np.ascontiguousarray(
                np.broadcast_to(bvb_row, (128, HPC * EP))),
            "wp": np.ascontiguousarray(
                W_proj[384 * g:384 * (g + 1), :]).astype(NPBF),
            "maskd": mask_const,
            "idnd": idn_const,
        })

    trace = bool(int(os.environ.get("KBENCH_TRACE", "0")))
    if trace:
        _install_ntff_hook()
    res = run_bass_kernel_spmd(
        nc, in_maps, list(range(N_CORES)), trace=trace,
    )
    kernel.last_exec_time_ns = res.exec_time_ns

    out = np.empty((B, T, C), dtype=np.float32)
    for b in range(B):
        out[b] = (res.results[2 * b]["yp"].astype(np.float32)
                  + res.results[2 * b + 1]["yp"].astype(np.float32) + b_proj)
    return out
